# revision 1
# baseline (speedup 1.0000x reference)
"""Trainium2 Bass kernel for EnhancedEdgeRankingGNN (gnn_message_passing).

Strategy (8 NeuronCores, SPMD):
  - Node-parallel GCN: core k owns nodes [k*6250,(k+1)*6250). Encoder + xw =
    h @ W computed locally, full xw tables assembled via AllGather;
    aggregation per dst-node-tile with one-hot "scatter matrices" S on the
    tensor engine (segment-sum as PSUM-accumulated matmul); self-loops are
    virtual edges with coeff dinv^2.
  - xw[src] rows fetched with the custom Q7 dma_gather (int16 indices =>
    tables split in two halves; host groups edges by src-half).
  - Global mean-pool partials per core -> AllReduce -> tiny graph MLP
    replicated.
  - Edge-parallel predictor MLP: core k owns edges [k*50000,(k+1)*50000).
    h[src]/h[dst] gathered from a bf16 AllGathered node table with
    dma_gather(transpose=True), landing directly in [feat, edge] layout;
    edge-attr encoder fused in SBUF; gf[batch[src]] applied via P = gf@ep1c
    and a one-hot matmul. LayerNorms use host-centered W3 (exact zero mean)
    + sum-of-squares matmul for variance.
  - Host work: index manipulation / layout prep only (bincount, grouping,
    padding, int16 index tables, weight reshuffling).
"""

import sys

sys.path.insert(0, "/opt/trn_rl_repo")

import numpy as np

N, E, G, H = 50000, 400000, 64, 128
NODE_IN, EDGE_IN = 3, 3
LN_EPS = 1e-5
NC = 8
NPC = N // NC            # 6250 nodes per core
NPCP = 6272              # padded to 49*128
TPC = NPCP // 128        # 49 dst tiles per core
ROWS = NC * NPCP         # 50176 padded table rows
HALFR = ROWS // 2        # 25088
EPC = E // NC            # 50000 edges per core
ET = 512                 # edge-MLP tile
GCALLN = 4096            # idxs per gcn gather call (single_packet=False)
GCALLE = 4096            # idxs per transpose gather call (needs single_packet=False)

bf16 = np.float16  # 16-bit storage dtype (fp16: more mantissa than bf16)


def _row_of_node(n):
    return (n // NPC) * NPCP + (n % NPC)


def _wrap_idx(a):
    """int16 index array -> [128, len/16] wrapped layout (replicated x8)."""
    assert len(a) % 16 == 0
    w = a.reshape(-1, 16).T  # [16, len/16]
    return np.tile(w, (8, 1)).astype(np.int16).copy()


def _center_w(w, b):
    """LN folding: (W - colmean, b - mean(b)) so mean over f of z is 0."""
    wc = w - w.mean(axis=1, keepdims=True)
    bc = b - b.mean()
    return wc.astype(np.float32), bc.astype(np.float32)


def preprocess(inputs):
    """Host-side index/layout prep. Returns (meta, data, reasm)."""
    x = np.asarray(inputs["x"], np.float32)
    ei = np.asarray(inputs["edge_index"])
    ea = np.asarray(inputs["edge_attr"], np.float32)
    batch = np.asarray(inputs["batch"]).astype(np.int64)
    src, dst = ei[0].astype(np.int64), ei[1].astype(np.int64)

    deg = np.bincount(dst, minlength=N).astype(np.float32) + 1.0
    dinv = (1.0 / np.sqrt(deg)).astype(np.float32)
    cnts = np.bincount(batch, minlength=G).astype(np.float32)
    inv_cnt = (1.0 / np.maximum(cnts, 1.0)).astype(np.float32)

    srcrow = _row_of_node(src)
    coeff_all = (dinv[src] * dinv[dst]).astype(np.float32)

    # ---------------- GCN edge structure (node-sharded by dst) -------------
    per_core_runs = []
    for k in range(NC):
        g0 = k * NPC
        sel = (dst >= g0) & (dst < g0 + NPC)
        s_r, d_l, c_e = srcrow[sel], (dst[sel] - g0), coeff_all[sel]
        own = np.arange(g0, g0 + NPC)
        s_r = np.concatenate([s_r, _row_of_node(own)])
        d_l = np.concatenate([d_l, own - g0])
        c_e = np.concatenate([c_e, (dinv[own] ** 2).astype(np.float32)])
        half = (s_r >= HALFR).astype(np.int64)
        tilei = d_l // 128
        runs = [[None] * TPC for _ in range(2)]
        for h in range(2):
            for t in range(TPC):
                m = (half == h) & (tilei == t)
                runs[h][t] = (
                    (s_r[m] - h * HALFR).astype(np.int16),
                    (d_l[m] % 128).astype(np.float32),
                    c_e[m].astype(np.float32),
                )
        per_core_runs.append(runs)

    rlp = [[0] * TPC for _ in range(2)]
    for h in range(2):
        for t in range(TPC):
            mx = max(len(per_core_runs[k][h][t][0]) for k in range(NC))
            rlp[h][t] = max(128, ((mx + 127) // 128) * 128)
    chunk_tile = []
    chunk_of_ht = {}
    half_sections = []
    c = 0
    for h in range(2):
        h0 = c
        for t in range(TPC):
            nch = rlp[h][t] // 128
            chunk_of_ht[(h, t)] = (c, nch)
            chunk_tile += [t] * nch
            c += nch
        half_sections.append((h0, c - h0))
    NCHUNK = c
    TOTG = NCHUNK * 128

    gcn_calls = []
    for h, (h0, hn) in enumerate(half_sections):
        s = h0 * 128
        end = (h0 + hn) * 128
        while s < end:
            n_ = min(GCALLN, end - s)
            gcn_calls.append((h, s, n_))
            s += n_

    gcn_idx_pc, gcn_dstloc_pc, gcn_coeff_pc = [], [], []
    for k in range(NC):
        lidx = np.zeros(TOTG, np.int16)
        dloc = np.zeros(TOTG, np.float32)
        cofs = np.zeros(TOTG, np.float32)
        for h in range(2):
            for t in range(TPC):
                c0, _ = chunk_of_ht[(h, t)]
                li, dl, ce = per_core_runs[k][h][t]
                s = c0 * 128
                lidx[s:s + len(li)] = li
                dloc[s:s + len(li)] = dl
                cofs[s:s + len(li)] = ce
        gcn_idx_pc.append(_wrap_idx(lidx))
        gcn_dstloc_pc.append(dloc.reshape(NCHUNK, 128).T.copy())
        gcn_coeff_pc.append(cofs.reshape(NCHUNK, 128).T.copy())

    # ---------------- edge-MLP structure (edge-sharded) --------------------
    dstrow = _row_of_node(dst)
    ebatch_all = batch[src].astype(np.float32)
    grp_all = 2 * (srcrow >= HALFR).astype(np.int64) + (dstrow >= HALFR)
    glp = [0] * 4
    orders, counts = [], []
    for k in range(NC):
        e0 = k * EPC
        g_e = grp_all[e0:e0 + EPC]
        order = np.argsort(g_e, kind="stable")
        cnt = np.bincount(g_e, minlength=4)
        orders.append(order)
        counts.append(cnt)
        for g in range(4):
            glp[g] = max(glp[g], ((int(cnt[g]) + ET - 1) // ET) * ET)
    goff = np.concatenate([[0], np.cumsum(glp)]).astype(np.int64)
    EP = int(goff[4])
    NT = EP // ET
    grp_of_tile = []
    for g in range(4):
        grp_of_tile += [g] * (glp[g] // ET)

    mlp_calls = []
    for g in range(4):
        s = int(goff[g])
        while s < goff[g + 1]:
            n_ = min(GCALLE, int(goff[g + 1]) - s)
            mlp_calls.append((s, n_))
            s += n_

    esrc_pc, edst_pc, ebatch_pc, attr_pc, pos_pc = [], [], [], [], []
    ohE_pc = []
    NTB = (NT + 2) // 3
    for k in range(NC):
        e0 = k * EPC
        order, cnt = orders[k], counts[k]
        si = np.zeros(EP, np.int64)
        valid = np.zeros(EP, bool)
        pos_of_local = np.empty(EPC, np.int64)
        cstart = np.cumsum(np.concatenate([[0], cnt]))
        for g in range(4):
            loc = order[cstart[g]:cstart[g + 1]]
            p0 = int(goff[g])
            si[p0:p0 + len(loc)] = e0 + loc
            valid[p0:p0 + len(loc)] = True
            pos_of_local[loc] = p0 + np.arange(len(loc))
        sr = srcrow[si]
        dr = dstrow[si]
        hs = (sr >= HALFR).astype(np.int64)
        hd = (dr >= HALFR).astype(np.int64)
        esrc_pc.append(_wrap_idx((sr - hs * HALFR).astype(np.int16)))
        edst_pc.append(_wrap_idx((dr - hd * HALFR).astype(np.int16)))
        eb = ebatch_all[si].copy()
        eb[~valid] = 0.0
        ebatch_pc.append(eb.astype(bf16).reshape(1, EP))
        ohE_pc.append((eb[None, :] == np.arange(G, dtype=np.float32)[:, None])
                      .astype(bf16))
        ap = np.zeros((128, NTB * ET), np.float32)
        av = ea[si].copy()
        av[~valid] = 0.0
        for t in range(NT):
            b = 32 * (t % 3)
            cb = (t // 3) * ET
            ap[b:b + EDGE_IN, cb:cb + ET] = av[t * ET:(t + 1) * ET].T
        attr_pc.append(ap.astype(bf16))
        pos_pc.append(pos_of_local)

    # ---------------- node-encoder inputs ----------------------------------
    NTA = (NPCP + ET - 1) // ET
    NTAB = (NTA + 2) // 3
    xpk_pc, bval_pc = [], []
    for k in range(NC):
        g0 = k * NPC
        xT = np.zeros((NODE_IN, NPCP), np.float32)
        xT[:, :NPC] = x[g0:g0 + NPC].T
        xp = np.zeros((128, NTAB * ET), np.float32)
        for t in range(NTA):
            b = 32 * (t % 3)
            cb = (t // 3) * ET
            wv = min(ET, NPCP - t * ET)
            xp[b:b + NODE_IN, cb:cb + wv] = xT[:, t * ET:t * ET + wv]
        xpk_pc.append(xp)
        bvflat = np.full(NPCP, -1.0, np.float32)
        bvflat[:NPC] = batch[g0:g0 + NPC].astype(np.float32)
        bval_pc.append(bvflat.reshape(TPC, 128).T.copy())

    meta = dict(
        NCHUNK=NCHUNK, chunk_tile=chunk_tile, chunk_of_ht=chunk_of_ht,
        gcn_calls=gcn_calls, half_sections=half_sections, rlp=rlp,
        EP=EP, NT=NT, NTB=NTB, grp_of_tile=grp_of_tile, mlp_calls=mlp_calls,
        NTA=NTA, NTAB=NTAB, TOTG=TOTG,
    )
    data = dict(
        inv_cnt=inv_cnt, gcn_idx=gcn_idx_pc, gcn_dstloc=gcn_dstloc_pc,
        gcn_coeff=gcn_coeff_pc, esrc=esrc_pc, edst=edst_pc, ebatch=ebatch_pc,
        attr=attr_pc, xpk=xpk_pc, bval=bval_pc, ohE=ohE_pc,
    )
    reasm = dict(pos=pos_pc)
    return meta, data, reasm


def prep_weights(inputs):
    f32 = np.float32
    w = {}

    def rep_small(wname, k_in):
        v = np.zeros((128, 128), f32)
        ww = np.asarray(inputs[wname], f32)
        for b in range(4):
            v[32 * b:32 * b + k_in, :] = ww
        return v

    w["ne1w"] = rep_small("ne1_w", NODE_IN)
    w["ne2w"] = np.asarray(inputs["ne2_w"], f32)
    ne3wc, ne3bc = _center_w(np.asarray(inputs["ne3_w"], f32),
                             np.asarray(inputs["ne3_b"], f32))
    w["ne3wc"] = ne3wc
    w["ne1b"] = np.asarray(inputs["ne1_b"], f32).reshape(128, 1)
    w["ne2b"] = np.asarray(inputs["ne2_b"], f32).reshape(128, 1)
    w["ne3bc"] = ne3bc.reshape(128, 1)
    w["neg"] = np.asarray(inputs["ne_g"], f32).reshape(128, 1)
    w["nebb"] = np.asarray(inputs["ne_bb"], f32).reshape(128, 1)

    w["ee1w"] = rep_small("ee1_w", EDGE_IN).astype(bf16)
    w["ee2w"] = np.asarray(inputs["ee2_w"], f32).astype(bf16)
    ee3wc, ee3bc = _center_w(np.asarray(inputs["ee3_w"], f32),
                             np.asarray(inputs["ee3_b"], f32))
    w["ee3wc"] = ee3wc.astype(bf16)
    w["ee1b"] = np.asarray(inputs["ee1_b"], f32).reshape(128, 1)
    w["ee2b"] = np.asarray(inputs["ee2_b"], f32).reshape(128, 1)
    w["ee3bc"] = ee3bc.reshape(128, 1)
    w["eeg"] = np.asarray(inputs["ee_g"], f32).reshape(128, 1)
    w["eebb"] = np.asarray(inputs["ee_bb"], f32).reshape(128, 1)

    w["g1w"] = np.asarray(inputs["g1_w"], f32)
    w["g2w"] = np.asarray(inputs["g2_w"], f32)
    w["g1b"] = np.asarray(inputs["g1_b"], f32).reshape(128, 1)
    w["g2bb"] = np.tile(np.asarray(inputs["g2_b"], f32)[None, :], (128, 1))

    w["gp1w"] = np.asarray(inputs["gp1_w"], f32)
    gp2wc, gp2bc = _center_w(np.asarray(inputs["gp2_w"], f32),
                             np.asarray(inputs["gp2_b"], f32))
    w["gp2wc"] = gp2wc
    w["gp1b"] = np.asarray(inputs["gp1_b"], f32).reshape(128, 1)
    w["gp2bc"] = gp2bc.reshape(128, 1)
    w["gpg"] = np.asarray(inputs["gp_g"], f32).reshape(128, 1)
    w["gpbb"] = np.asarray(inputs["gp_bb"], f32).reshape(128, 1)

    ep1 = np.asarray(inputs["ep1_w"], f32)
    w["ep1a"] = ep1[0:128].astype(bf16)
    w["ep1b"] = ep1[128:256].astype(bf16)
    w["ep1c"] = ep1[256:384].copy()
    w["ep1d"] = ep1[384:512].astype(bf16)
    w["ep1bias"] = np.asarray(inputs["ep1_b"], f32).reshape(2, 128).T.copy()
    ep2 = np.asarray(inputs["ep2_w"], f32)
    w["ep2w"] = np.concatenate([ep2[0:128], ep2[128:256]], axis=1).astype(bf16)
    w["ep2b"] = np.asarray(inputs["ep2_b"], f32).reshape(128, 1)
    w["ep3w"] = np.asarray(inputs["ep3_w"], f32).astype(bf16)
    w["ep3b"] = np.asarray(inputs["ep3_b"], f32).reshape(64, 1)
    w["ep4w"] = np.asarray(inputs["ep4_w"], f32).astype(bf16)
    w["ep4b"] = np.asarray(inputs["ep4_b"], f32).reshape(1, 1).copy()

    w["iota128"] = np.broadcast_to(np.arange(128, dtype=f32)[None, :],
                                   (128, 128)).copy()
    w["ones1"] = np.ones((1, 128), f32)
    w["ones1h"] = np.ones((1, 64), np.float16)
    w["iotap"] = np.arange(128, dtype=f32).reshape(128, 1).copy()
    w["ones_over_f"] = np.full((128, 128), 1.0 / 128.0, f32)
    return w


# keys that stay HBM-resident or get custom SBUF handling
_NO_CONST = {"gcn_idx", "esrc", "edst", "ohE"}


def build_program(meta, w, data_shapes):
    import os as _os
    PHASE = int(_os.environ.get("K_PHASE", "4"))
    NTLIM = int(_os.environ.get("K_NTLIM", "0"))
    NOGATH = int(_os.environ.get("K_NOGATH", "0"))
    NOPB = int(_os.environ.get("K_NOPB", "0"))
    import concourse.bacc as bacc
    import concourse.mybir as mybir
    import concourse.tile as tile

    f32, b16, i16 = mybir.dt.float32, mybir.dt.float16, mybir.dt.int16
    AF = mybir.ActivationFunctionType
    ALU = mybir.AluOpType

    NCHUNK, NT, EP, NTB = meta["NCHUNK"], meta["NT"], meta["EP"], meta["NTB"]
    NTA, NTAB, TOTG = meta["NTA"], meta["NTAB"], meta["TOTG"]
    chunk_of_ht = meta["chunk_of_ht"]
    gcn_calls = meta["gcn_calls"]
    mlp_calls = meta["mlp_calls"]
    grp_of_tile = meta["grp_of_tile"]

    nc = bacc.Bacc("TRN2", target_bir_lowering=False, debug=False,
                   num_devices=NC)

    t_in = {}
    for nm, arr in w.items():
        if isinstance(arr, np.ndarray):
            dt = b16 if arr.dtype == bf16 else f32
            t_in[nm] = nc.dram_tensor(nm, list(arr.shape), dt,
                                      kind="ExternalInput")
    for nm, (shape, dt_s) in data_shapes.items():
        dt = {"f32": f32, "b16": b16, "i16": i16}[dt_s]
        t_in[nm] = nc.dram_tensor(nm, list(shape), dt, kind="ExternalInput")

    out_d = nc.dram_tensor("out", [1, EP], f32, kind="ExternalOutput")
    rg = [list(range(NC))]

    with tile.TileContext(nc) as tc:
        from contextlib import ExitStack
        with ExitStack() as ctx:
            cpool = ctx.enter_context(tc.tile_pool(name="consts", bufs=1))
            dram = ctx.enter_context(tc.tile_pool(name="dram", bufs=1,
                                                  space="DRAM"))
            ps = ctx.enter_context(tc.tile_pool(name="ps", bufs=7,
                                                space="PSUM"))
            work = ctx.enter_context(tc.tile_pool(name="work", bufs=2))
            big = ctx.enter_context(tc.tile_pool(name="big", bufs=1))
            efT_pool = ctx.enter_context(tc.tile_pool(name="efT", bufs=10))
            zc_pool = ctx.enter_context(tc.tile_pool(name="zc", bufs=3))

            # ---- constants into SBUF
            c_sb = {}
            for nm, t in t_in.items():
                if nm in _NO_CONST:
                    continue
                tile_ = cpool.tile(list(t.shape), t.dtype, tag=f"c_{nm}")
                nc.sync.dma_start(tile_[:], t.ap())
                c_sb[nm] = tile_

            def C(nm):
                return c_sb[nm][:]

            # ---- DRAM scratch
            xw1_own = dram.tile([NPCP, H], b16)
            xw1_full = dram.tile([ROWS, H], b16)
            xw2_own = dram.tile([NPCP, H], b16)
            xw2_full = dram.tile([ROWS, H], b16)
            h2b_own = dram.tile([NPCP, H], b16)
            h2b_full = dram.tile([ROWS, H], b16)
            xw1_b = dram.tile([HALFR, H], b16)
            xw2_b = dram.tile([HALFR, H], b16)
            h2b_b = dram.tile([HALFR, H], b16)
            ar_in = dram.tile([128, G], f32)
            ar_out = dram.tile([128, G], f32)

            # ---- small persistent SBUF
            sums_acc = big.tile([128, G], f32, tag="sums")
            gfT = big.tile([128, G], f32, tag="gfT")
            P_sb = big.tile([64, 256], b16, tag="P")
            esrc_sb = big.tile([128, EP // 16], i16, tag="esrc")
            nc.sync.dma_start(esrc_sb[:], t_in["esrc"].ap())
            edst_sb = big.tile([128, EP // 16], i16, tag="edst")
            nc.sync.dma_start(edst_sb[:], t_in["edst"].ap())

            nc.vector.memset(sums_acc[:], 0.0)

            # ---- LayerNorm tail helper (z centered, [128, wv] f32 in SBUF)
            def ln_tail(lnp, z_ap, wv, gname, bbname, out_ap):
                sq = lnp.tile([128, ET], f32, tag="ln_sq")
                nc.scalar.activation(sq[:, :wv], z_ap, AF.Square)
                # all-ones lhsT -> every output row holds the column mean-sq:
                # the variance arrives already partition-broadcast.
                msp = ps.tile([128, ET], f32, tag="ps")
                nc.tensor.matmul(msp[:, :wv], C("ones_over_f"), sq[:, :wv],
                                 start=True, stop=True)
                msv = lnp.tile([128, ET], f32, tag="ln_ms")
                nc.vector.tensor_scalar(msv[:, :wv], msp[:, :wv], LN_EPS,
                                        None, ALU.add)
                nc.vector.reciprocal(msv[:, :wv], msv[:, :wv])
                rstd = lnp.tile([128, ET], f32, tag="ln_rs")
                nc.scalar.activation(rstd[:, :wv], msv[:, :wv], AF.Sqrt)
                rstdb = lnp.tile([128, ET], f32, tag="ln_rb")
                nc.vector.tensor_mul(rstdb[:, :wv], z_ap, rstd[:, :wv])
                nc.scalar.activation(out_ap, rstdb[:, :wv], AF.Identity,
                                     bias=C(bbname), scale=C(gname))

            # ================= phase A: node encoder + xw1 =================
            with tc.tile_pool(name="pA", bufs=2) as pa, \
                 tc.tile_pool(name="pAbig", bufs=1) as pabig:
                h0T = pabig.tile([128, NPCP], f32, tag="h0T")
                for t in range(NTA):
                    b = 32 * (t % 3)
                    cb = (t // 3) * ET
                    wv = min(ET, NPCP - t * ET)
                    a_ap = c_sb["xpk"][b:b + NODE_IN, cb:cb + wv]
                    z1p = ps.tile([128, ET], f32, tag="ps")
                    nc.tensor.matmul(z1p[:, :wv],
                                     c_sb["ne1w"][b:b + NODE_IN, :], a_ap,
                                     start=True, stop=True)
                    z1s = pa.tile([128, ET], f32, tag="nz1")
                    nc.vector.tensor_scalar(z1s[:, :wv], z1p[:, :wv],
                                            C("ne1b"), 0.0, ALU.add, ALU.max)
                    z2p = ps.tile([128, ET], f32, tag="ps")
                    nc.tensor.matmul(z2p[:, :wv], C("ne2w"), z1s[:, :wv],
                                     start=True, stop=True)
                    z2s = pa.tile([128, ET], f32, tag="nz2")
                    nc.vector.tensor_scalar(z2s[:, :wv], z2p[:, :wv],
                                            C("ne2b"), 0.0, ALU.add, ALU.max)
                    z3p = ps.tile([128, ET], f32, tag="ps")
                    nc.tensor.matmul(z3p[:, :wv], C("ne3wc"), z2s[:, :wv],
                                     start=True, stop=True)
                    z3s = pa.tile([128, ET], f32, tag="nz3")
                    nc.vector.tensor_scalar(z3s[:, :wv], z3p[:, :wv],
                                            C("ne3bc"), None, ALU.add)
                    ln_tail(pa, z3s[:, :wv], wv, "neg", "nebb",
                            h0T[:, t * ET:t * ET + wv])

                for t in range(TPC):
                    xp = ps.tile([128, ET], f32, tag="ps")
                    nc.tensor.matmul(xp[:, :H],
                                     h0T[:, t * 128:(t + 1) * 128],
                                     C("g1w"), start=True, stop=True)
                    xs = work.tile([128, H], b16, tag="xw_sb")
                    nc.vector.tensor_copy(xs[:], xp[:, :H])
                    nc.sync.dma_start(xw1_own[t * 128:(t + 1) * 128, :],
                                      xs[:])
            nc.gpsimd.collective_compute(
                "AllGather", ALU.bypass, replica_groups=rg,
                ins=[xw1_own[:]], outs=[xw1_full[:]])
            nc.sync.dma_start(xw1_b[:], xw1_full[HALFR:ROWS, :])

            # ================= GCN layers =================
            call_of_chunk = {}
            for (hcall, s, n_) in gcn_calls:
                call_of_chunk[s // 128] = (hcall, s, n_)

            with tc.tile_pool(name="pB", bufs=1) as pb, \
                 tc.tile_pool(name="gcn_g", bufs=2) as gpool, \
                 tc.tile_pool(name="spool", bufs=4) as spool:
                h1T = pb.tile([128, NPCP], f32, tag="h1T")
                aggA = pb.tile([128, NPCP], f32, tag="aggA")
                gidx_sb = pb.tile([128, TOTG // 16], i16, tag="gidx")
                nc.sync.dma_start(gidx_sb[:], t_in["gcn_idx"].ap())

                def gcn_layer(layer, table_full):
                    cur = {"buf": None, "start": 0}

                    def ensure_gather(c):
                        if c in call_of_chunk:
                            hcall, s, n_ = call_of_chunk[c]
                            gb = gpool.tile([128, GCALLN // 128, H], b16,
                                            tag="gb")
                            view = (table_full[0][0:HALFR, :] if hcall == 0
                                    else table_full[1][:])
                            nc.gpsimd.dma_gather(
                                gb[:, :n_ // 128, :], view,
                                gidx_sb[:, s // 16:(s + n_) // 16],
                                n_, n_, H, single_packet=False)
                            cur["buf"] = gb
                            cur["start"] = c

                    for h in range(2):
                        for t in range(TPC):
                            c0, nch = chunk_of_ht[(h, t)]
                            pst = ps.tile([128, ET], f32, tag="ps")
                            for j in range(nch):
                                c = c0 + j
                                ensure_gather(c)
                                S = spool.tile([128, 128], b16, tag="S")
                                nc.vector.tensor_scalar(
                                    S[:], C("iota128"),
                                    c_sb["gcn_dstloc"][:, c:c + 1],
                                    c_sb["gcn_coeff"][:, c:c + 1],
                                    ALU.is_equal, ALU.mult)
                                gsl = cur["buf"][:, c - cur["start"], :]
                                if layer == 0:
                                    nc.tensor.matmul(pst[:, :128], gsl, S[:],
                                                     start=(j == 0),
                                                     stop=(j == nch - 1))
                                else:
                                    nc.tensor.matmul(pst[:, :128], S[:], gsl,
                                                     start=(j == 0),
                                                     stop=(j == nch - 1))
                            sl = aggA[:, t * 128:(t + 1) * 128]
                            if h == 0:
                                nc.vector.tensor_copy(sl, pst[:, :128])
                                continue
                            nc.vector.tensor_add(sl, sl, pst[:, :128])
                            if layer == 0:
                                nc.scalar.activation(
                                    h1T[:, t * 128:(t + 1) * 128], sl,
                                    AF.Relu, bias=C("g1b"))
                                xp = ps.tile([128, ET], f32, tag="ps")
                                nc.tensor.matmul(
                                    xp[:, :H],
                                    h1T[:, t * 128:(t + 1) * 128],
                                    C("g2w"), start=True, stop=True)
                                xs = work.tile([128, H], b16, tag="xw_sb")
                                nc.vector.tensor_copy(xs[:], xp[:, :H])
                                nc.sync.dma_start(
                                    xw2_own[t * 128:(t + 1) * 128, :], xs[:])
                            else:
                                h2t = work.tile([128, H], f32, tag="h2t")
                                nc.vector.tensor_add(h2t[:], sl, C("g2bb"))
                                ohb = work.tile([128, G], f32, tag="ohb")
                                nc.vector.tensor_scalar(
                                    ohb[:], c_sb["iota128"][:, 0:G],
                                    c_sb["bval"][:, t:t + 1], None,
                                    ALU.is_equal)
                                pp = ps.tile([128, ET], f32, tag="ps")
                                nc.tensor.matmul(pp[:, :G], h2t[:], ohb[:],
                                                 start=True, stop=True)
                                nc.vector.tensor_add(sums_acc[:],
                                                     sums_acc[:], pp[:, :G])
                                h2b = work.tile([128, H], b16, tag="h2b")
                                nc.vector.tensor_copy(h2b[:], h2t[:])
                                nc.sync.dma_start(
                                    h2b_own[t * 128:(t + 1) * 128, :],
                                    h2b[:])

                if PHASE >= 2:
                    gcn_layer(0, (xw1_full, xw1_b))
                if PHASE >= 3:
                    nc.gpsimd.collective_compute(
                        "AllGather", ALU.bypass, replica_groups=rg,
                        ins=[xw2_own[:]], outs=[xw2_full[:]])
                    nc.sync.dma_start(xw2_b[:], xw2_full[HALFR:ROWS, :])
                    gcn_layer(1, (xw2_full, xw2_b))
                    nc.gpsimd.collective_compute(
                        "AllGather", ALU.bypass, replica_groups=rg,
                        ins=[h2b_own[:]], outs=[h2b_full[:]])
                    nc.sync.dma_start(h2b_b[:], h2b_full[HALFR:ROWS, :])

            lnpC = ctx.enter_context(tc.tile_pool(name="lnC", bufs=2))
            if PHASE >= 3:
                # ================= graph MLP (replicated) =================
                nc.sync.dma_start(ar_in[:], sums_acc[:])
                nc.gpsimd.collective_compute(
                    "AllReduce", ALU.add, replica_groups=rg,
                    ins=[ar_in[:]], outs=[ar_out[:]])
                sums_sb = work.tile([128, G], f32, tag="sums_sb")
                nc.sync.dma_start(sums_sb[:], ar_out[:])
                icb = work.tile([128, G], f32, tag="icb")
                nc.gpsimd.partition_broadcast(icb[:], c_sb["inv_cnt"][0:1, :])
                gm = work.tile([128, G], f32, tag="gm")
                nc.vector.tensor_mul(gm[:], sums_sb[:], icb[:])
                z1p = ps.tile([128, ET], f32, tag="ps")
                nc.tensor.matmul(z1p[:, :G], C("gp1w"), gm[:], start=True,
                                 stop=True)
                gf1 = work.tile([128, G], f32, tag="gf1")
                nc.scalar.activation(gf1[:], z1p[:, :G], AF.Relu, bias=C("gp1b"))
                z2p = ps.tile([128, ET], f32, tag="ps")
                nc.tensor.matmul(z2p[:, :G], C("gp2wc"), gf1[:], start=True,
                                 stop=True)
                z2c = work.tile([128, G], f32, tag="z2c")
                nc.vector.tensor_scalar(z2c[:], z2p[:, :G], C("gp2bc"), None,
                                        ALU.add)
                ln_tail(lnpC, z2c[:], G, "gpg", "gpbb", gfT[:])
                Pp = ps.tile([128, ET], f32, tag="ps")
                nc.tensor.matmul(Pp[:64, :256], gfT[:], C("ep1c"), start=True,
                                 stop=True)
                nc.vector.tensor_copy(P_sb[:], Pp[:64, :256])

            if PHASE >= 4:
                # ================= phase C: edge MLP =================
                c_call_of_tile = {}
                for (s, n_) in mlp_calls:
                    c_call_of_tile[s // ET] = (s, n_)

                with tc.tile_pool(name="gsrc", bufs=2) as gs_pool, \
                     tc.tile_pool(name="gdst", bufs=2) as gd_pool, \
                     tc.tile_pool(name="ebt", bufs=3) as eb_pool:
                    cbuf = {"s": None, "d": None, "start": 0}
                    for t in range(NTLIM if NTLIM else NT):
                        grp = grp_of_tile[t]
                        hs, hd = grp >> 1, grp & 1
                        if t in c_call_of_tile:
                            s, n_ = c_call_of_tile[t]
                            gsb = gs_pool.tile([128, 1, GCALLE], b16, tag="gs")
                            gdb = gd_pool.tile([128, 1, GCALLE], b16, tag="gd")
                            vs = (h2b_full[0:HALFR, :] if hs == 0
                                  else h2b_full[HALFR:ROWS, :])
                            vd = (h2b_full[0:HALFR, :] if hd == 0
                                  else h2b_full[HALFR:ROWS, :])
                            if NOGATH:
                                nc.vector.memset(gsb[:], 0.5)
                                nc.vector.memset(gdb[:], 0.5)
                            else:
                                nc.gpsimd.dma_gather(
                                    gsb[:, :, :n_], vs,
                                    esrc_sb[:, s // 16:(s + n_) // 16], n_, n_, H,
                                    transpose=True, single_packet=False)
                                nc.gpsimd.dma_gather(
                                    gdb[:, :, :n_], vd,
                                    edst_sb[:, s // 16:(s + n_) // 16], n_, n_, H,
                                    transpose=True, single_packet=False)
                            cbuf["s"], cbuf["d"] = gsb, gdb
                            cbuf["start"] = s
                        off = t * ET - cbuf["start"]
                        src_sl = cbuf["s"][:, 0, off:off + ET]
                        dst_sl = cbuf["d"][:, 0, off:off + ET]

                        # edge-attr encoder
                        b = 32 * (t % 3)
                        cb = (t // 3) * ET
                        a_ap = c_sb["attr"][b:b + EDGE_IN, cb:cb + ET]
                        z1p = ps.tile([128, ET], f32, tag="ps")
                        nc.tensor.matmul(z1p[:], c_sb["ee1w"][b:b + EDGE_IN, :],
                                         a_ap, start=True, stop=True)
                        z1s = zc_pool.tile([128, ET], b16, tag="ez1")
                        nc.vector.tensor_scalar(z1s[:], z1p[:], C("ee1b"), 0.0,
                                                ALU.add, ALU.max)
                        z2p = ps.tile([128, ET], f32, tag="ps")
                        nc.tensor.matmul(z2p[:], C("ee2w"), z1s[:], start=True,
                                         stop=True)
                        z2s = zc_pool.tile([128, ET], b16, tag="ez2")
                        nc.vector.tensor_scalar(z2s[:], z2p[:], C("ee2b"), 0.0,
                                                ALU.add, ALU.max)
                        z3p = ps.tile([128, ET], f32, tag="ps")
                        nc.tensor.matmul(z3p[:], C("ee3wc"), z2s[:], start=True,
                                         stop=True)
                        z3s = zc_pool.tile([128, ET], f32, tag="ez3")
                        nc.vector.tensor_scalar(z3s[:], z3p[:], C("ee3bc"), None,
                                                ALU.add)
                        eft = efT_pool.tile([128, ET], b16, tag="eft")
                        ln_tail(lnpC, z3s[:], ET, "eeg", "eebb", eft[:])

                        # gf one-hot (host-precomputed), 4 tiles per DMA
                        if t % 4 == 0:
                            ohw = min(4, (NTLIM if NTLIM else NT) - t) * ET
                            oh4 = eb_pool.tile([64, 4 * ET], b16, tag="oht")  # noqa
                            nc.sync.dma_start(
                                oh4[:, :ohw],
                                t_in["ohE"].ap()[0:G, t * ET:t * ET + ohw])
                            cbuf["oh4"] = oh4
                        oh = cbuf["oh4"][:, (t % 4) * ET:(t % 4 + 1) * ET]

                        # L1
                        z1sb = []
                        for mc in range(2):
                            zp = ps.tile([128, ET], f32, tag="ps")
                            m0 = mc * 128
                            nc.tensor.matmul(zp[:], c_sb["ep1a"][:, m0:m0 + 128],
                                             src_sl, start=True, stop=False)
                            nc.tensor.matmul(zp[:], c_sb["ep1b"][:, m0:m0 + 128],
                                             dst_sl, start=False, stop=False)
                            nc.tensor.matmul(zp[:], c_sb["ep1d"][:, m0:m0 + 128],
                                             eft[:], start=False, stop=False)
                            nc.tensor.matmul(zp[:], P_sb[:, m0:m0 + 128],
                                             oh, start=False, stop=True)
                            zs = zc_pool.tile([128, ET], b16, tag=f"z1_{mc}")
                            nc.scalar.activation(
                                zs[:], zp[:], AF.Tanh,
                                bias=c_sb["ep1bias"][:, mc:mc + 1])
                            z1sb.append(zs)

                        # L2
                        z2pp = ps.tile([128, ET], f32, tag="ps")
                        for kc in range(2):
                            nc.tensor.matmul(
                                z2pp[:], c_sb["ep2w"][:, kc * 128:kc * 128 + 128],
                                z1sb[kc][:], start=(kc == 0), stop=(kc == 1))
                        z2sb = zc_pool.tile([128, ET], b16, tag="z2")
                        nc.scalar.activation(z2sb[:], z2pp[:], AF.Tanh,
                                             bias=C("ep2b"))

                        # L3
                        z3pp = ps.tile([128, ET], f32, tag="ps")
                        nc.tensor.matmul(z3pp[:64, :], C("ep3w"), z2sb[:],
                                         start=True, stop=True)
                        z3sb = zc_pool.tile([64, ET], b16, tag="z3")
                        nc.vector.tensor_scalar(z3sb[:], z3pp[:64, :], C("ep3b"),
                                                0.0, ALU.add, ALU.max)

                        # L4 + sigmoid
                        z4p = ps.tile([128, ET], f32, tag="ps")
                        nc.tensor.matmul(z4p[:1, :], C("ep4w"), z3sb[:],
                                         start=True, stop=True)
                        if t % 4 == 0:
                            ob4_t = eb_pool.tile([1, 4 * ET], f32, tag="os4")
                            cbuf["ob4"] = ob4_t
                        ob4 = cbuf["ob4"]
                        nc.scalar.activation(
                            ob4[0:1, (t % 4) * ET:(t % 4 + 1) * ET],
                            z4p[:1, :], AF.Sigmoid, bias=C("ep4b"))
                        ntl = NTLIM if NTLIM else NT
                        if t % 4 == 3 or t == ntl - 1:
                            t0b = (t // 4) * 4
                            wv_o = (t - t0b + 1) * ET
                            nc.sync.dma_start(
                                out_d.ap()[0:1, t0b * ET:t0b * ET + wv_o],
                                ob4[0:1, :wv_o])

    nc.compile()
    return nc


def _data_shapes(meta, data):
    i16, b16s, f32s = "i16", "b16", "f32"
    return {
        "inv_cnt": ([1, G], f32s),
        "xpk": (list(data["xpk"][0].shape), f32s),
        "bval": ([128, TPC], f32s),
        "gcn_idx": (list(data["gcn_idx"][0].shape), i16),
        "gcn_dstloc": ([128, meta["NCHUNK"]], f32s),
        "gcn_coeff": ([128, meta["NCHUNK"]], f32s),
        "esrc": (list(data["esrc"][0].shape), i16),
        "edst": (list(data["edst"][0].shape), i16),
        "ohE": ([G, meta["EP"]], b16s),
        "attr": (list(data["attr"][0].shape), b16s),
    }


def kernel(**inputs) -> np.ndarray:
    from concourse.bass_utils import run_bass_kernel_spmd

    meta, data, reasm = preprocess(inputs)
    w = prep_weights(inputs)
    nc = build_program(meta, w, _data_shapes(meta, data))

    in_maps = []
    for k in range(NC):
        m = {nm: arr for nm, arr in w.items() if isinstance(arr, np.ndarray)}
        m["inv_cnt"] = data["inv_cnt"].reshape(1, G)
        m["xpk"] = data["xpk"][k]
        m["bval"] = data["bval"][k]
        m["gcn_idx"] = data["gcn_idx"][k]
        m["gcn_dstloc"] = data["gcn_dstloc"][k]
        m["gcn_coeff"] = data["gcn_coeff"][k]
        m["esrc"] = data["esrc"][k]
        m["edst"] = data["edst"][k]
        m["ohE"] = data["ohE"][k]
        m["attr"] = data["attr"][k]
        in_maps.append(m)

    res = run_bass_kernel_spmd(nc, in_maps, core_ids=list(range(NC)))
    globals()["LAST_RESULTS"] = res

    import os as _os, time as _time
    nbench = int(_os.environ.get("K_BENCH", "0"))
    if nbench:
        times = []
        for _ in range(nbench):
            t0 = _time.time()
            run_bass_kernel_spmd(nc, in_maps, core_ids=list(range(NC)))
            times.append(_time.time() - t0)
        globals()["LAST_BENCH"] = times

    out = np.empty((E, 1), np.float32)
    for k in range(NC):
        oc = np.asarray(res.results[k]["out"]).reshape(-1)
        e0 = k * EPC
        out[e0:e0 + EPC, 0] = oc[reasm["pos"][k]]
    return out



# revision 3
# speedup vs baseline: 31.8743x; 31.8743x over previous
"""Trainium2 Bass kernel for EnhancedEdgeRankingGNN (gnn_message_passing).

Strategy (8 NeuronCores, SPMD):
  - Node-parallel GCN: core k owns nodes [k*6250,(k+1)*6250). Encoder + xw =
    h @ W computed locally, full xw tables assembled via AllGather;
    aggregation per dst-node-tile with one-hot "scatter matrices" S on the
    tensor engine (segment-sum as PSUM-accumulated matmul); self-loops are
    virtual edges with coeff dinv^2.
  - xw[src] rows fetched with the custom Q7 dma_gather (int16 indices =>
    tables split in two halves; host groups edges by src-half).
  - Global mean-pool partials per core -> AllReduce -> tiny graph MLP
    replicated.
  - Edge-parallel predictor MLP: core k owns edges [k*50000,(k+1)*50000).
    h[src]/h[dst] gathered from a bf16 AllGathered node table with
    dma_gather(transpose=True), landing directly in [feat, edge] layout;
    edge-attr encoder fused in SBUF; gf[batch[src]] applied via P = gf@ep1c
    and a one-hot matmul. LayerNorms use host-centered W3 (exact zero mean)
    + sum-of-squares matmul for variance.
  - Host work: index manipulation / layout prep only (bincount, grouping,
    padding, int16 index tables, weight reshuffling).
"""

import sys

sys.path.insert(0, "/opt/trn_rl_repo")

import numpy as np

N, E, G, H = 50000, 400000, 64, 128
NODE_IN, EDGE_IN = 3, 3
LN_EPS = 1e-5
NC = 8
NPC = N // NC            # 6250 nodes per core
NPCP = 6272              # padded to 49*128
TPC = NPCP // 128        # 49 dst tiles per core
ROWS = NC * NPCP         # 50176 padded table rows
HALFR = ROWS // 2        # 25088
EPC = E // NC            # 50000 edges per core
ET = 512                 # edge-MLP tile
GCALLN = 4096            # idxs per gcn gather call (single_packet=False)
GCALLE = 4096            # idxs per transpose gather call (needs single_packet=False)

bf16 = np.float16  # 16-bit storage dtype (fp16: more mantissa than bf16)


def _row_of_node(n):
    return (n // NPC) * NPCP + (n % NPC)


def _wrap_idx(a):
    """int16 index array -> [128, len/16] wrapped layout (replicated x8)."""
    assert len(a) % 16 == 0
    w = a.reshape(-1, 16).T  # [16, len/16]
    return np.tile(w, (8, 1)).astype(np.int16).copy()


def _center_w(w, b):
    """LN folding: (W - colmean, b - mean(b)) so mean over f of z is 0."""
    wc = w - w.mean(axis=1, keepdims=True)
    bc = b - b.mean()
    return wc.astype(np.float32), bc.astype(np.float32)


def preprocess(inputs):
    """Host-side index/layout prep. Returns (meta, data, reasm)."""
    x = np.asarray(inputs["x"], np.float32)
    ei = np.asarray(inputs["edge_index"])
    ea = np.asarray(inputs["edge_attr"], np.float32)
    batch = np.asarray(inputs["batch"]).astype(np.int64)
    src, dst = ei[0].astype(np.int64), ei[1].astype(np.int64)

    deg = np.bincount(dst, minlength=N).astype(np.float32) + 1.0
    dinv = (1.0 / np.sqrt(deg)).astype(np.float32)
    cnts = np.bincount(batch, minlength=G).astype(np.float32)
    inv_cnt = (1.0 / np.maximum(cnts, 1.0)).astype(np.float32)

    srcrow = _row_of_node(src)
    coeff_all = (dinv[src] * dinv[dst]).astype(np.float32)

    # ---------------- GCN edge structure (node-sharded by dst) -------------
    per_core_runs = []
    for k in range(NC):
        g0 = k * NPC
        sel = (dst >= g0) & (dst < g0 + NPC)
        s_r, d_l, c_e = srcrow[sel], (dst[sel] - g0), coeff_all[sel]
        own = np.arange(g0, g0 + NPC)
        s_r = np.concatenate([s_r, _row_of_node(own)])
        d_l = np.concatenate([d_l, own - g0])
        c_e = np.concatenate([c_e, (dinv[own] ** 2).astype(np.float32)])
        half = (s_r >= HALFR).astype(np.int64)
        tilei = d_l // 128
        runs = [[None] * TPC for _ in range(2)]
        for h in range(2):
            for t in range(TPC):
                m = (half == h) & (tilei == t)
                runs[h][t] = (
                    (s_r[m] - h * HALFR).astype(np.int16),
                    (d_l[m] % 128).astype(np.float32),
                    c_e[m].astype(np.float32),
                )
        per_core_runs.append(runs)

    rlp = [[0] * TPC for _ in range(2)]
    for h in range(2):
        for t in range(TPC):
            mx = max(len(per_core_runs[k][h][t][0]) for k in range(NC))
            rlp[h][t] = max(128, ((mx + 127) // 128) * 128)
    chunk_tile = []
    chunk_of_ht = {}
    half_sections = []
    c = 0
    for h in range(2):
        h0 = c
        for t in range(TPC):
            nch = rlp[h][t] // 128
            chunk_of_ht[(h, t)] = (c, nch)
            chunk_tile += [t] * nch
            c += nch
        half_sections.append((h0, c - h0))
    NCHUNK = c
    TOTG = NCHUNK * 128

    gcn_calls = []
    for h, (h0, hn) in enumerate(half_sections):
        s = h0 * 128
        end = (h0 + hn) * 128
        while s < end:
            n_ = min(GCALLN, end - s)
            gcn_calls.append((h, s, n_))
            s += n_

    gcn_idx_pc, gcn_dstloc_pc, gcn_coeff_pc = [], [], []
    for k in range(NC):
        lidx = np.zeros(TOTG, np.int16)
        dloc = np.zeros(TOTG, np.float32)
        cofs = np.zeros(TOTG, np.float32)
        for h in range(2):
            for t in range(TPC):
                c0, _ = chunk_of_ht[(h, t)]
                li, dl, ce = per_core_runs[k][h][t]
                s = c0 * 128
                lidx[s:s + len(li)] = li
                dloc[s:s + len(li)] = dl
                cofs[s:s + len(li)] = ce
        gcn_idx_pc.append(_wrap_idx(lidx))
        gcn_dstloc_pc.append(dloc.reshape(NCHUNK, 128).T.copy())
        gcn_coeff_pc.append(cofs.reshape(NCHUNK, 128).T.copy())

    # ---------------- edge-MLP structure (edge-sharded) --------------------
    dstrow = _row_of_node(dst)
    ebatch_all = batch[src].astype(np.float32)
    grp_all = 2 * (srcrow >= HALFR).astype(np.int64) + (dstrow >= HALFR)
    glp = [0] * 4
    orders, counts = [], []
    for k in range(NC):
        e0 = k * EPC
        g_e = grp_all[e0:e0 + EPC]
        order = np.argsort(g_e, kind="stable")
        cnt = np.bincount(g_e, minlength=4)
        orders.append(order)
        counts.append(cnt)
        for g in range(4):
            glp[g] = max(glp[g], ((int(cnt[g]) + ET - 1) // ET) * ET)
    goff = np.concatenate([[0], np.cumsum(glp)]).astype(np.int64)
    EP = int(goff[4])
    NT = EP // ET
    grp_of_tile = []
    for g in range(4):
        grp_of_tile += [g] * (glp[g] // ET)

    mlp_calls = []
    for g in range(4):
        s = int(goff[g])
        while s < goff[g + 1]:
            n_ = min(GCALLE, int(goff[g + 1]) - s)
            mlp_calls.append((s, n_))
            s += n_

    esrc_pc, edst_pc, ebatch_pc, attr_pc, pos_pc = [], [], [], [], []
    ohE_pc = []
    NTB = (NT + 2) // 3
    for k in range(NC):
        e0 = k * EPC
        order, cnt = orders[k], counts[k]
        si = np.zeros(EP, np.int64)
        valid = np.zeros(EP, bool)
        pos_of_local = np.empty(EPC, np.int64)
        cstart = np.cumsum(np.concatenate([[0], cnt]))
        for g in range(4):
            loc = order[cstart[g]:cstart[g + 1]]
            p0 = int(goff[g])
            si[p0:p0 + len(loc)] = e0 + loc
            valid[p0:p0 + len(loc)] = True
            pos_of_local[loc] = p0 + np.arange(len(loc))
        sr = srcrow[si]
        dr = dstrow[si]
        hs = (sr >= HALFR).astype(np.int64)
        hd = (dr >= HALFR).astype(np.int64)
        esrc_pc.append(_wrap_idx((sr - hs * HALFR).astype(np.int16)))
        edst_pc.append(_wrap_idx((dr - hd * HALFR).astype(np.int16)))
        eb = ebatch_all[si].copy()
        eb[~valid] = 0.0
        ebatch_pc.append(eb.astype(bf16).reshape(1, EP))
        ohE_pc.append((eb[None, :] == np.arange(G, dtype=np.float32)[:, None])
                      .astype(bf16))
        ap = np.zeros((128, NTB * ET), np.float32)
        av = ea[si].copy()
        av[~valid] = 0.0
        for t in range(NT):
            b = 32 * (t % 3)
            cb = (t // 3) * ET
            ap[b:b + EDGE_IN, cb:cb + ET] = av[t * ET:(t + 1) * ET].T
        attr_pc.append(ap.astype(bf16))
        pos_pc.append(pos_of_local)

    # ---------------- node-encoder inputs ----------------------------------
    NTA = (NPCP + ET - 1) // ET
    NTAB = (NTA + 2) // 3
    xpk_pc, bval_pc = [], []
    for k in range(NC):
        g0 = k * NPC
        xT = np.zeros((NODE_IN, NPCP), np.float32)
        xT[:, :NPC] = x[g0:g0 + NPC].T
        xp = np.zeros((128, NTAB * ET), np.float32)
        for t in range(NTA):
            b = 32 * (t % 3)
            cb = (t // 3) * ET
            wv = min(ET, NPCP - t * ET)
            xp[b:b + NODE_IN, cb:cb + wv] = xT[:, t * ET:t * ET + wv]
        xpk_pc.append(xp)
        bvflat = np.full(NPCP, -1.0, np.float32)
        bvflat[:NPC] = batch[g0:g0 + NPC].astype(np.float32)
        bval_pc.append(bvflat.reshape(TPC, 128).T.copy())

    meta = dict(
        NCHUNK=NCHUNK, chunk_tile=chunk_tile, chunk_of_ht=chunk_of_ht,
        gcn_calls=gcn_calls, half_sections=half_sections, rlp=rlp,
        EP=EP, NT=NT, NTB=NTB, grp_of_tile=grp_of_tile, mlp_calls=mlp_calls,
        NTA=NTA, NTAB=NTAB, TOTG=TOTG,
    )
    data = dict(
        inv_cnt=inv_cnt, gcn_idx=gcn_idx_pc, gcn_dstloc=gcn_dstloc_pc,
        gcn_coeff=gcn_coeff_pc, esrc=esrc_pc, edst=edst_pc, ebatch=ebatch_pc,
        attr=attr_pc, xpk=xpk_pc, bval=bval_pc, ohE=ohE_pc,
    )
    reasm = dict(pos=pos_pc)
    return meta, data, reasm


def prep_weights(inputs):
    f32 = np.float32
    w = {}

    def rep_small(wname, k_in):
        v = np.zeros((128, 128), f32)
        ww = np.asarray(inputs[wname], f32)
        for b in range(4):
            v[32 * b:32 * b + k_in, :] = ww
        return v

    w["ne1w"] = rep_small("ne1_w", NODE_IN)
    w["ne2w"] = np.asarray(inputs["ne2_w"], f32)
    ne3wc, ne3bc = _center_w(np.asarray(inputs["ne3_w"], f32),
                             np.asarray(inputs["ne3_b"], f32))
    w["ne3wc"] = ne3wc
    w["ne1b"] = np.asarray(inputs["ne1_b"], f32).reshape(128, 1)
    w["ne2b"] = np.asarray(inputs["ne2_b"], f32).reshape(128, 1)
    w["ne3bc"] = ne3bc.reshape(128, 1)
    w["neg"] = np.asarray(inputs["ne_g"], f32).reshape(128, 1)
    w["nebb"] = np.asarray(inputs["ne_bb"], f32).reshape(128, 1)

    w["ee1w"] = rep_small("ee1_w", EDGE_IN).astype(bf16)
    w["ee2w"] = np.asarray(inputs["ee2_w"], f32).astype(bf16)
    ee3wc, ee3bc = _center_w(np.asarray(inputs["ee3_w"], f32),
                             np.asarray(inputs["ee3_b"], f32))
    w["ee3wc"] = ee3wc.astype(bf16)
    w["ee1b"] = np.asarray(inputs["ee1_b"], f32).reshape(128, 1)
    w["ee2b"] = np.asarray(inputs["ee2_b"], f32).reshape(128, 1)
    w["ee3bc"] = ee3bc.reshape(128, 1)
    w["eeg"] = np.asarray(inputs["ee_g"], f32).reshape(128, 1)
    w["eebb"] = np.asarray(inputs["ee_bb"], f32).reshape(128, 1)

    w["g1w"] = np.asarray(inputs["g1_w"], f32)
    w["g2w"] = np.asarray(inputs["g2_w"], f32)
    w["g1b"] = np.asarray(inputs["g1_b"], f32).reshape(128, 1)
    w["g2bb"] = np.tile(np.asarray(inputs["g2_b"], f32)[None, :], (128, 1))

    w["gp1w"] = np.asarray(inputs["gp1_w"], f32)
    gp2wc, gp2bc = _center_w(np.asarray(inputs["gp2_w"], f32),
                             np.asarray(inputs["gp2_b"], f32))
    w["gp2wc"] = gp2wc
    w["gp1b"] = np.asarray(inputs["gp1_b"], f32).reshape(128, 1)
    w["gp2bc"] = gp2bc.reshape(128, 1)
    w["gpg"] = np.asarray(inputs["gp_g"], f32).reshape(128, 1)
    w["gpbb"] = np.asarray(inputs["gp_bb"], f32).reshape(128, 1)

    ep1 = np.asarray(inputs["ep1_w"], f32)
    w["ep1a"] = ep1[0:128].astype(bf16)
    w["ep1b"] = ep1[128:256].astype(bf16)
    w["ep1c"] = ep1[256:384].copy()
    w["ep1d"] = ep1[384:512].astype(bf16)
    w["ep1bias"] = np.asarray(inputs["ep1_b"], f32).reshape(2, 128).T.copy()
    ep2 = np.asarray(inputs["ep2_w"], f32)
    w["ep2w"] = np.concatenate([ep2[0:128], ep2[128:256]], axis=1).astype(bf16)
    w["ep2b"] = np.asarray(inputs["ep2_b"], f32).reshape(128, 1)
    w["ep3w"] = np.asarray(inputs["ep3_w"], f32).astype(bf16)
    w["ep3b"] = np.asarray(inputs["ep3_b"], f32).reshape(64, 1)
    w["ep4w"] = np.asarray(inputs["ep4_w"], f32).astype(bf16)
    w["ep4b"] = np.asarray(inputs["ep4_b"], f32).reshape(1, 1).copy()

    w["iota128"] = np.broadcast_to(np.arange(128, dtype=f32)[None, :],
                                   (128, 128)).copy()
    w["ones1"] = np.ones((1, 128), f32)
    w["ones1h"] = np.ones((1, 64), np.float16)
    w["iotap"] = np.arange(128, dtype=f32).reshape(128, 1).copy()
    w["ones_over_f"] = np.full((128, 128), 1.0 / 128.0, f32)
    return w


# keys that stay HBM-resident or get custom SBUF handling
_NO_CONST = {"gcn_idx", "esrc", "edst", "ohE"}


def build_program(meta, w, data_shapes):
    import os as _os
    PHASE = int(_os.environ.get("K_PHASE", "4"))
    NTLIM = int(_os.environ.get("K_NTLIM", "0"))
    NOGATH = int(_os.environ.get("K_NOGATH", "0"))
    NOPB = int(_os.environ.get("K_NOPB", "0"))
    import concourse.bacc as bacc
    import concourse.mybir as mybir
    import concourse.tile as tile

    f32, b16, i16 = mybir.dt.float32, mybir.dt.float16, mybir.dt.int16
    AF = mybir.ActivationFunctionType
    ALU = mybir.AluOpType

    NCHUNK, NT, EP, NTB = meta["NCHUNK"], meta["NT"], meta["EP"], meta["NTB"]
    NTA, NTAB, TOTG = meta["NTA"], meta["NTAB"], meta["TOTG"]
    chunk_of_ht = meta["chunk_of_ht"]
    gcn_calls = meta["gcn_calls"]
    mlp_calls = meta["mlp_calls"]
    grp_of_tile = meta["grp_of_tile"]

    nc = bacc.Bacc("TRN2", target_bir_lowering=False, debug=False,
                   num_devices=NC)

    t_in = {}
    for nm, arr in w.items():
        if isinstance(arr, np.ndarray):
            dt = b16 if arr.dtype == bf16 else f32
            t_in[nm] = nc.dram_tensor(nm, list(arr.shape), dt,
                                      kind="ExternalInput")
    for nm, (shape, dt_s) in data_shapes.items():
        dt = {"f32": f32, "b16": b16, "i16": i16}[dt_s]
        t_in[nm] = nc.dram_tensor(nm, list(shape), dt, kind="ExternalInput")

    out_d = nc.dram_tensor("out", [1, EP], f32, kind="ExternalOutput")
    rg = [list(range(NC))]

    with tile.TileContext(nc) as tc:
        from contextlib import ExitStack
        with ExitStack() as ctx:
            cpool = ctx.enter_context(tc.tile_pool(name="consts", bufs=1))
            dram = ctx.enter_context(tc.tile_pool(name="dram", bufs=1,
                                                  space="DRAM"))
            ps = ctx.enter_context(tc.tile_pool(name="ps", bufs=7,
                                                space="PSUM"))
            work = ctx.enter_context(tc.tile_pool(name="work", bufs=2))
            big = ctx.enter_context(tc.tile_pool(name="big", bufs=1))
            efT_pool = ctx.enter_context(tc.tile_pool(name="efT", bufs=10))
            zc_pool = ctx.enter_context(tc.tile_pool(name="zc", bufs=3))

            # ---- constants into SBUF
            c_sb = {}
            for nm, t in t_in.items():
                if nm in _NO_CONST:
                    continue
                tile_ = cpool.tile(list(t.shape), t.dtype, tag=f"c_{nm}")
                nc.sync.dma_start(tile_[:], t.ap())
                c_sb[nm] = tile_

            def C(nm):
                return c_sb[nm][:]

            # ---- DRAM scratch
            xw1_own = dram.tile([NPCP, H], b16)
            xw1_full = dram.tile([ROWS, H], b16)
            xw2_own = dram.tile([NPCP, H], b16)
            xw2_full = dram.tile([ROWS, H], b16)
            h2b_own = dram.tile([NPCP, H], b16)
            h2b_full = dram.tile([ROWS, H], b16)
            xw1_b = dram.tile([HALFR, H], b16)
            xw2_b = dram.tile([HALFR, H], b16)
            h2b_b = dram.tile([HALFR, H], b16)
            ar_in = dram.tile([128, G], f32)
            ar_out = dram.tile([128, G], f32)

            # ---- small persistent SBUF
            sums_acc = big.tile([128, G], f32, tag="sums")
            gfT = big.tile([128, G], f32, tag="gfT")
            P_sb = big.tile([64, 256], b16, tag="P")
            esrc_sb = big.tile([128, EP // 16], i16, tag="esrc")
            nc.sync.dma_start(esrc_sb[:], t_in["esrc"].ap())
            edst_sb = big.tile([128, EP // 16], i16, tag="edst")
            nc.sync.dma_start(edst_sb[:], t_in["edst"].ap())

            nc.vector.memset(sums_acc[:], 0.0)

            # ---- LayerNorm tail helper (z centered, [128, wv] f32 in SBUF)
            def ln_tail(lnp, z_ap, wv, gname, bbname, out_ap):
                sq = lnp.tile([128, ET], f32, tag="ln_sq")
                nc.scalar.activation(sq[:, :wv], z_ap, AF.Square)
                # all-ones lhsT -> every output row holds the column mean-sq:
                # the variance arrives already partition-broadcast.
                msp = ps.tile([128, ET], f32, tag="ps")
                nc.tensor.matmul(msp[:, :wv], C("ones_over_f"), sq[:, :wv],
                                 start=True, stop=True)
                msv = lnp.tile([128, ET], f32, tag="ln_ms")
                nc.vector.tensor_scalar(msv[:, :wv], msp[:, :wv], LN_EPS,
                                        None, ALU.add)
                nc.vector.reciprocal(msv[:, :wv], msv[:, :wv])
                rstd = lnp.tile([128, ET], f32, tag="ln_rs")
                nc.scalar.activation(rstd[:, :wv], msv[:, :wv], AF.Sqrt)
                rstdb = lnp.tile([128, ET], f32, tag="ln_rb")
                nc.vector.tensor_mul(rstdb[:, :wv], z_ap, rstd[:, :wv])
                nc.scalar.activation(out_ap, rstdb[:, :wv], AF.Identity,
                                     bias=C(bbname), scale=C(gname))

            # ================= phase A: node encoder + xw1 =================
            with tc.tile_pool(name="pA", bufs=2) as pa, \
                 tc.tile_pool(name="pAbig", bufs=1) as pabig:
                h0T = pabig.tile([128, NPCP], f32, tag="h0T")
                for t in range(NTA):
                    b = 32 * (t % 3)
                    cb = (t // 3) * ET
                    wv = min(ET, NPCP - t * ET)
                    a_ap = c_sb["xpk"][b:b + NODE_IN, cb:cb + wv]
                    z1p = ps.tile([128, ET], f32, tag="ps")
                    nc.tensor.matmul(z1p[:, :wv],
                                     c_sb["ne1w"][b:b + NODE_IN, :], a_ap,
                                     start=True, stop=True)
                    z1s = pa.tile([128, ET], f32, tag="nz1")
                    nc.vector.tensor_scalar(z1s[:, :wv], z1p[:, :wv],
                                            C("ne1b"), 0.0, ALU.add, ALU.max)
                    z2p = ps.tile([128, ET], f32, tag="ps")
                    nc.tensor.matmul(z2p[:, :wv], C("ne2w"), z1s[:, :wv],
                                     start=True, stop=True)
                    z2s = pa.tile([128, ET], f32, tag="nz2")
                    nc.vector.tensor_scalar(z2s[:, :wv], z2p[:, :wv],
                                            C("ne2b"), 0.0, ALU.add, ALU.max)
                    z3p = ps.tile([128, ET], f32, tag="ps")
                    nc.tensor.matmul(z3p[:, :wv], C("ne3wc"), z2s[:, :wv],
                                     start=True, stop=True)
                    z3s = pa.tile([128, ET], f32, tag="nz3")
                    nc.vector.tensor_scalar(z3s[:, :wv], z3p[:, :wv],
                                            C("ne3bc"), None, ALU.add)
                    ln_tail(pa, z3s[:, :wv], wv, "neg", "nebb",
                            h0T[:, t * ET:t * ET + wv])

                for t in range(TPC):
                    xp = ps.tile([128, ET], f32, tag="ps")
                    nc.tensor.matmul(xp[:, :H],
                                     h0T[:, t * 128:(t + 1) * 128],
                                     C("g1w"), start=True, stop=True)
                    xs = work.tile([128, H], b16, tag="xw_sb")
                    nc.vector.tensor_copy(xs[:], xp[:, :H])
                    nc.sync.dma_start(xw1_own[t * 128:(t + 1) * 128, :],
                                      xs[:])
            nc.gpsimd.collective_compute(
                "AllGather", ALU.bypass, replica_groups=rg,
                ins=[xw1_own[:]], outs=[xw1_full[:]])
            nc.sync.dma_start(xw1_b[:], xw1_full[HALFR:ROWS, :])

            # ================= GCN layers =================
            call_of_chunk = {}
            for (hcall, s, n_) in gcn_calls:
                call_of_chunk[s // 128] = (hcall, s, n_)

            with tc.tile_pool(name="pB", bufs=1) as pb, \
                 tc.tile_pool(name="gcn_g", bufs=2) as gpool, \
                 tc.tile_pool(name="spool", bufs=4) as spool:
                h1T = pb.tile([128, NPCP], f32, tag="h1T")
                aggA = pb.tile([128, NPCP], f32, tag="aggA")
                gidx_sb = pb.tile([128, TOTG // 16], i16, tag="gidx")
                nc.sync.dma_start(gidx_sb[:], t_in["gcn_idx"].ap())

                def gcn_layer(layer, table_full):
                    cur = {"buf": None, "start": 0}

                    def ensure_gather(c):
                        if c in call_of_chunk:
                            hcall, s, n_ = call_of_chunk[c]
                            gb = gpool.tile([128, GCALLN // 128, H], b16,
                                            tag="gb")
                            view = (table_full[0][0:HALFR, :] if hcall == 0
                                    else table_full[1][:])
                            nc.gpsimd.dma_gather(
                                gb[:, :n_ // 128, :], view,
                                gidx_sb[:, s // 16:(s + n_) // 16],
                                n_, n_, H, single_packet=False)
                            cur["buf"] = gb
                            cur["start"] = c

                    for h in range(2):
                        for t in range(TPC):
                            c0, nch = chunk_of_ht[(h, t)]
                            pst = ps.tile([128, ET], f32, tag="ps")
                            for j in range(nch):
                                c = c0 + j
                                ensure_gather(c)
                                S = spool.tile([128, 128], b16, tag="S")
                                nc.vector.tensor_scalar(
                                    S[:], C("iota128"),
                                    c_sb["gcn_dstloc"][:, c:c + 1],
                                    c_sb["gcn_coeff"][:, c:c + 1],
                                    ALU.is_equal, ALU.mult)
                                gsl = cur["buf"][:, c - cur["start"], :]
                                if layer == 0:
                                    nc.tensor.matmul(pst[:, :128], gsl, S[:],
                                                     start=(j == 0),
                                                     stop=(j == nch - 1))
                                else:
                                    nc.tensor.matmul(pst[:, :128], S[:], gsl,
                                                     start=(j == 0),
                                                     stop=(j == nch - 1))
                            sl = aggA[:, t * 128:(t + 1) * 128]
                            if h == 0:
                                nc.vector.tensor_copy(sl, pst[:, :128])
                                continue
                            nc.vector.tensor_add(sl, sl, pst[:, :128])
                            if layer == 0:
                                nc.scalar.activation(
                                    h1T[:, t * 128:(t + 1) * 128], sl,
                                    AF.Relu, bias=C("g1b"))
                                xp = ps.tile([128, ET], f32, tag="ps")
                                nc.tensor.matmul(
                                    xp[:, :H],
                                    h1T[:, t * 128:(t + 1) * 128],
                                    C("g2w"), start=True, stop=True)
                                xs = work.tile([128, H], b16, tag="xw_sb")
                                nc.vector.tensor_copy(xs[:], xp[:, :H])
                                nc.sync.dma_start(
                                    xw2_own[t * 128:(t + 1) * 128, :], xs[:])
                            else:
                                h2t = work.tile([128, H], f32, tag="h2t")
                                nc.vector.tensor_add(h2t[:], sl, C("g2bb"))
                                ohb = work.tile([128, G], f32, tag="ohb")
                                nc.vector.tensor_scalar(
                                    ohb[:], c_sb["iota128"][:, 0:G],
                                    c_sb["bval"][:, t:t + 1], None,
                                    ALU.is_equal)
                                pp = ps.tile([128, ET], f32, tag="ps")
                                nc.tensor.matmul(pp[:, :G], h2t[:], ohb[:],
                                                 start=True, stop=True)
                                nc.vector.tensor_add(sums_acc[:],
                                                     sums_acc[:], pp[:, :G])
                                h2b = work.tile([128, H], b16, tag="h2b")
                                nc.vector.tensor_copy(h2b[:], h2t[:])
                                nc.sync.dma_start(
                                    h2b_own[t * 128:(t + 1) * 128, :],
                                    h2b[:])

                if PHASE >= 2:
                    gcn_layer(0, (xw1_full, xw1_b))
                if PHASE >= 3:
                    nc.gpsimd.collective_compute(
                        "AllGather", ALU.bypass, replica_groups=rg,
                        ins=[xw2_own[:]], outs=[xw2_full[:]])
                    nc.sync.dma_start(xw2_b[:], xw2_full[HALFR:ROWS, :])
                    gcn_layer(1, (xw2_full, xw2_b))
                    nc.gpsimd.collective_compute(
                        "AllGather", ALU.bypass, replica_groups=rg,
                        ins=[h2b_own[:]], outs=[h2b_full[:]])
                    nc.sync.dma_start(h2b_b[:], h2b_full[HALFR:ROWS, :])

            lnpC = ctx.enter_context(tc.tile_pool(name="lnC", bufs=2))
            if PHASE >= 3:
                # ================= graph MLP (replicated) =================
                nc.sync.dma_start(ar_in[:], sums_acc[:])
                nc.gpsimd.collective_compute(
                    "AllReduce", ALU.add, replica_groups=rg,
                    ins=[ar_in[:]], outs=[ar_out[:]])
                sums_sb = work.tile([128, G], f32, tag="sums_sb")
                nc.sync.dma_start(sums_sb[:], ar_out[:])
                icb = work.tile([128, G], f32, tag="icb")
                nc.gpsimd.partition_broadcast(icb[:], c_sb["inv_cnt"][0:1, :])
                gm = work.tile([128, G], f32, tag="gm")
                nc.vector.tensor_mul(gm[:], sums_sb[:], icb[:])
                z1p = ps.tile([128, ET], f32, tag="ps")
                nc.tensor.matmul(z1p[:, :G], C("gp1w"), gm[:], start=True,
                                 stop=True)
                gf1 = work.tile([128, G], f32, tag="gf1")
                nc.scalar.activation(gf1[:], z1p[:, :G], AF.Relu, bias=C("gp1b"))
                z2p = ps.tile([128, ET], f32, tag="ps")
                nc.tensor.matmul(z2p[:, :G], C("gp2wc"), gf1[:], start=True,
                                 stop=True)
                z2c = work.tile([128, G], f32, tag="z2c")
                nc.vector.tensor_scalar(z2c[:], z2p[:, :G], C("gp2bc"), None,
                                        ALU.add)
                ln_tail(lnpC, z2c[:], G, "gpg", "gpbb", gfT[:])
                Pp = ps.tile([128, ET], f32, tag="ps")
                nc.tensor.matmul(Pp[:64, :256], gfT[:], C("ep1c"), start=True,
                                 stop=True)
                nc.vector.tensor_copy(P_sb[:], Pp[:64, :256])

            if PHASE >= 4:
                # ================= phase C: edge MLP =================
                c_call_of_tile = {}
                for (s, n_) in mlp_calls:
                    c_call_of_tile[s // ET] = (s, n_)

                with tc.tile_pool(name="gsrc", bufs=2) as gs_pool, \
                     tc.tile_pool(name="gdst", bufs=2) as gd_pool, \
                     tc.tile_pool(name="ebt", bufs=3) as eb_pool:
                    cbuf = {"s": None, "d": None, "start": 0}
                    for t in range(NTLIM if NTLIM else NT):
                        grp = grp_of_tile[t]
                        hs, hd = grp >> 1, grp & 1
                        if t in c_call_of_tile:
                            s, n_ = c_call_of_tile[t]
                            gsb = gs_pool.tile([128, 1, GCALLE], b16, tag="gs")
                            gdb = gd_pool.tile([128, 1, GCALLE], b16, tag="gd")
                            vs = (h2b_full[0:HALFR, :] if hs == 0
                                  else h2b_full[HALFR:ROWS, :])
                            vd = (h2b_full[0:HALFR, :] if hd == 0
                                  else h2b_full[HALFR:ROWS, :])
                            if NOGATH:
                                nc.vector.memset(gsb[:], 0.5)
                                nc.vector.memset(gdb[:], 0.5)
                            else:
                                nc.gpsimd.dma_gather(
                                    gsb[:, :, :n_], vs,
                                    esrc_sb[:, s // 16:(s + n_) // 16], n_, n_, H,
                                    transpose=True, single_packet=False)
                                nc.gpsimd.dma_gather(
                                    gdb[:, :, :n_], vd,
                                    edst_sb[:, s // 16:(s + n_) // 16], n_, n_, H,
                                    transpose=True, single_packet=False)
                            cbuf["s"], cbuf["d"] = gsb, gdb
                            cbuf["start"] = s
                        off = t * ET - cbuf["start"]
                        src_sl = cbuf["s"][:, 0, off:off + ET]
                        dst_sl = cbuf["d"][:, 0, off:off + ET]

                        # edge-attr encoder
                        b = 32 * (t % 3)
                        cb = (t // 3) * ET
                        a_ap = c_sb["attr"][b:b + EDGE_IN, cb:cb + ET]
                        z1p = ps.tile([128, ET], f32, tag="ps")
                        nc.tensor.matmul(z1p[:], c_sb["ee1w"][b:b + EDGE_IN, :],
                                         a_ap, start=True, stop=True)
                        z1s = zc_pool.tile([128, ET], b16, tag="ez1")
                        nc.vector.tensor_scalar(z1s[:], z1p[:], C("ee1b"), 0.0,
                                                ALU.add, ALU.max)
                        z2p = ps.tile([128, ET], f32, tag="ps")
                        nc.tensor.matmul(z2p[:], C("ee2w"), z1s[:], start=True,
                                         stop=True)
                        z2s = zc_pool.tile([128, ET], b16, tag="ez2")
                        nc.vector.tensor_scalar(z2s[:], z2p[:], C("ee2b"), 0.0,
                                                ALU.add, ALU.max)
                        z3p = ps.tile([128, ET], f32, tag="ps")
                        nc.tensor.matmul(z3p[:], C("ee3wc"), z2s[:], start=True,
                                         stop=True)
                        z3s = zc_pool.tile([128, ET], f32, tag="ez3")
                        nc.vector.tensor_scalar(z3s[:], z3p[:], C("ee3bc"), None,
                                                ALU.add)
                        eft = efT_pool.tile([128, ET], b16, tag="eft")
                        ln_tail(lnpC, z3s[:], ET, "eeg", "eebb", eft[:])

                        # gf one-hot (host-precomputed), 4 tiles per DMA
                        if t % 4 == 0:
                            ohw = min(4, (NTLIM if NTLIM else NT) - t) * ET
                            oh4 = eb_pool.tile([64, 4 * ET], b16, tag="oht")  # noqa
                            nc.sync.dma_start(
                                oh4[:, :ohw],
                                t_in["ohE"].ap()[0:G, t * ET:t * ET + ohw])
                            cbuf["oh4"] = oh4
                        oh = cbuf["oh4"][:, (t % 4) * ET:(t % 4 + 1) * ET]

                        # L1
                        z1sb = []
                        for mc in range(2):
                            zp = ps.tile([128, ET], f32, tag="ps")
                            m0 = mc * 128
                            nc.tensor.matmul(zp[:], c_sb["ep1a"][:, m0:m0 + 128],
                                             src_sl, start=True, stop=False)
                            nc.tensor.matmul(zp[:], c_sb["ep1b"][:, m0:m0 + 128],
                                             dst_sl, start=False, stop=False)
                            nc.tensor.matmul(zp[:], c_sb["ep1d"][:, m0:m0 + 128],
                                             eft[:], start=False, stop=False)
                            nc.tensor.matmul(zp[:], P_sb[:, m0:m0 + 128],
                                             oh, start=False, stop=True)
                            zs = zc_pool.tile([128, ET], b16, tag=f"z1_{mc}")
                            nc.scalar.activation(
                                zs[:], zp[:], AF.Tanh,
                                bias=c_sb["ep1bias"][:, mc:mc + 1])
                            z1sb.append(zs)

                        # L2
                        z2pp = ps.tile([128, ET], f32, tag="ps")
                        for kc in range(2):
                            nc.tensor.matmul(
                                z2pp[:], c_sb["ep2w"][:, kc * 128:kc * 128 + 128],
                                z1sb[kc][:], start=(kc == 0), stop=(kc == 1))
                        z2sb = zc_pool.tile([128, ET], b16, tag="z2")
                        nc.scalar.activation(z2sb[:], z2pp[:], AF.Tanh,
                                             bias=C("ep2b"))

                        # L3
                        z3pp = ps.tile([128, ET], f32, tag="ps")
                        nc.tensor.matmul(z3pp[:64, :], C("ep3w"), z2sb[:],
                                         start=True, stop=True)
                        z3sb = zc_pool.tile([64, ET], b16, tag="z3")
                        nc.vector.tensor_scalar(z3sb[:], z3pp[:64, :], C("ep3b"),
                                                0.0, ALU.add, ALU.max)

                        # L4 + sigmoid
                        z4p = ps.tile([128, ET], f32, tag="ps")
                        nc.tensor.matmul(z4p[:1, :], C("ep4w"), z3sb[:],
                                         start=True, stop=True)
                        if t % 4 == 0:
                            ob4_t = eb_pool.tile([1, 4 * ET], f32, tag="os4")
                            cbuf["ob4"] = ob4_t
                        ob4 = cbuf["ob4"]
                        nc.scalar.activation(
                            ob4[0:1, (t % 4) * ET:(t % 4 + 1) * ET],
                            z4p[:1, :], AF.Sigmoid, bias=C("ep4b"))
                        ntl = NTLIM if NTLIM else NT
                        if t % 4 == 3 or t == ntl - 1:
                            t0b = (t // 4) * 4
                            wv_o = (t - t0b + 1) * ET
                            nc.sync.dma_start(
                                out_d.ap()[0:1, t0b * ET:t0b * ET + wv_o],
                                ob4[0:1, :wv_o])

    nc.compile()
    return nc


def _data_shapes(meta, data):
    i16, b16s, f32s = "i16", "b16", "f32"
    return {
        "inv_cnt": ([1, G], f32s),
        "xpk": (list(data["xpk"][0].shape), f32s),
        "bval": ([128, TPC], f32s),
        "gcn_idx": (list(data["gcn_idx"][0].shape), i16),
        "gcn_dstloc": ([128, meta["NCHUNK"]], f32s),
        "gcn_coeff": ([128, meta["NCHUNK"]], f32s),
        "esrc": (list(data["esrc"][0].shape), i16),
        "edst": (list(data["edst"][0].shape), i16),
        "ohE": ([G, meta["EP"]], b16s),
        "attr": (list(data["attr"][0].shape), b16s),
    }


def build_all(inputs):
    """Build program + per-core input maps. Shared by kernel() and bench."""
    meta, data, reasm = preprocess(inputs)
    w = prep_weights(inputs)
    nc = build_program(meta, w, _data_shapes(meta, data))

    in_maps = []
    for k in range(NC):
        m = {nm: arr for nm, arr in w.items() if isinstance(arr, np.ndarray)}
        m["inv_cnt"] = data["inv_cnt"].reshape(1, G)
        m["xpk"] = data["xpk"][k]
        m["bval"] = data["bval"][k]
        m["gcn_idx"] = data["gcn_idx"][k]
        m["gcn_dstloc"] = data["gcn_dstloc"][k]
        m["gcn_coeff"] = data["gcn_coeff"][k]
        m["esrc"] = data["esrc"][k]
        m["edst"] = data["edst"][k]
        m["ohE"] = data["ohE"][k]
        m["attr"] = data["attr"][k]
        in_maps.append(m)
    return nc, in_maps, meta, reasm


def kernel(**inputs) -> np.ndarray:
    from concourse.bass_utils import run_bass_kernel_spmd

    nc, in_maps, meta, reasm = build_all(inputs)

    import os as _os0
    _tr = bool(int(_os0.environ.get("K_TRACE", "0")))
    _kw = {}
    if _tr:
        _kw["trace"] = True
        _td = _os0.environ.get("K_TMPDIR")
        if _td:
            _kw["tmpdir"] = _td
        _tc = _os0.environ.get("K_TRACE_CORES")
        if _tc:
            _kw["trace_cores"] = [int(c) for c in _tc.split(",")]
    res = run_bass_kernel_spmd(nc, in_maps, core_ids=list(range(NC)), **_kw)
    globals()["LAST_RESULTS"] = res

    import os as _os, time as _time
    nbench = int(_os.environ.get("K_BENCH", "0"))
    if nbench:
        times = []
        for _ in range(nbench):
            t0 = _time.time()
            run_bass_kernel_spmd(nc, in_maps, core_ids=list(range(NC)))
            times.append(_time.time() - t0)
        globals()["LAST_BENCH"] = times

    out = np.empty((E, 1), np.float32)
    for k in range(NC):
        oc = np.asarray(res.results[k]["out"]).reshape(-1)
        e0 = k * EPC
        out[e0:e0 + EPC, 0] = oc[reasm["pos"][k]]
    return out



# revision 7
# speedup vs baseline: 32.0108x; 1.0043x over previous
"""Trainium2 Bass kernel for EnhancedEdgeRankingGNN (gnn_message_passing).

Strategy (8 NeuronCores, SPMD):
  - Node-parallel GCN: core k owns nodes [k*6250,(k+1)*6250). Encoder + xw =
    h @ W computed locally, full xw tables assembled via AllGather;
    aggregation per dst-node-tile with one-hot "scatter matrices" S on the
    tensor engine (segment-sum as PSUM-accumulated matmul); self-loops are
    virtual edges with coeff dinv^2.
  - xw[src] rows fetched with the custom Q7 dma_gather (int16 indices =>
    tables split in two halves; host groups edges by src-half).
  - Global mean-pool partials per core -> AllReduce -> tiny graph MLP
    replicated.
  - Edge-parallel predictor MLP: core k owns edges [k*50000,(k+1)*50000).
    h[src]/h[dst] gathered from a bf16 AllGathered node table with
    dma_gather(transpose=True), landing directly in [feat, edge] layout;
    edge-attr encoder fused in SBUF; gf[batch[src]] applied via P = gf@ep1c
    and a one-hot matmul. LayerNorms use host-centered W3 (exact zero mean)
    + sum-of-squares matmul for variance.
  - Host work: index manipulation / layout prep only (bincount, grouping,
    padding, int16 index tables, weight reshuffling).
"""

import sys

sys.path.insert(0, "/opt/trn_rl_repo")

import numpy as np

N, E, G, H = 50000, 400000, 64, 128
NODE_IN, EDGE_IN = 3, 3
LN_EPS = 1e-5
NC = 8
NPC = N // NC            # 6250 nodes per core
NPCP = 6272              # padded to 49*128
TPC = NPCP // 128        # 49 dst tiles per core
ROWS = NC * NPCP         # 50176 padded table rows
HALFR = ROWS // 2        # 25088
EPC = E // NC            # 50000 edges per core
ET = 512                 # edge-MLP tile
GCALLN = 4096            # idxs per gcn gather call (single_packet=False)
GCALLE = 4096            # idxs per transpose gather call (needs single_packet=False)

bf16 = np.float16  # 16-bit storage dtype (fp16: more mantissa than bf16)


def _row_of_node(n):
    return (n // NPC) * NPCP + (n % NPC)


def _wrap_idx(a):
    """int16 index array -> [128, len/16] wrapped layout (replicated x8)."""
    assert len(a) % 16 == 0
    w = a.reshape(-1, 16).T  # [16, len/16]
    return np.tile(w, (8, 1)).astype(np.int16).copy()


def _center_w(w, b):
    """LN folding: (W - colmean, b - mean(b)) so mean over f of z is 0."""
    wc = w - w.mean(axis=1, keepdims=True)
    bc = b - b.mean()
    return wc.astype(np.float32), bc.astype(np.float32)


def preprocess(inputs):
    """Host-side index/layout prep. Returns (meta, data, reasm)."""
    x = np.asarray(inputs["x"], np.float32)
    ei = np.asarray(inputs["edge_index"])
    ea = np.asarray(inputs["edge_attr"], np.float32)
    batch = np.asarray(inputs["batch"]).astype(np.int64)
    src, dst = ei[0].astype(np.int64), ei[1].astype(np.int64)

    deg = np.bincount(dst, minlength=N).astype(np.float32) + 1.0
    dinv = (1.0 / np.sqrt(deg)).astype(np.float32)
    cnts = np.bincount(batch, minlength=G).astype(np.float32)
    inv_cnt = (1.0 / np.maximum(cnts, 1.0)).astype(np.float32)

    srcrow = _row_of_node(src)
    coeff_all = (dinv[src] * dinv[dst]).astype(np.float32)

    # ---------------- GCN edge structure (node-sharded by dst) -------------
    per_core_runs = []
    for k in range(NC):
        g0 = k * NPC
        sel = (dst >= g0) & (dst < g0 + NPC)
        s_r, d_l, c_e = srcrow[sel], (dst[sel] - g0), coeff_all[sel]
        own = np.arange(g0, g0 + NPC)
        s_r = np.concatenate([s_r, _row_of_node(own)])
        d_l = np.concatenate([d_l, own - g0])
        c_e = np.concatenate([c_e, (dinv[own] ** 2).astype(np.float32)])
        half = (s_r >= HALFR).astype(np.int64)
        tilei = d_l // 128
        runs = [[None] * TPC for _ in range(2)]
        for h in range(2):
            for t in range(TPC):
                m = (half == h) & (tilei == t)
                runs[h][t] = (
                    (s_r[m] - h * HALFR).astype(np.int16),
                    (d_l[m] % 128).astype(np.float32),
                    c_e[m].astype(np.float32),
                )
        per_core_runs.append(runs)

    rlp = [[0] * TPC for _ in range(2)]
    for h in range(2):
        for t in range(TPC):
            mx = max(len(per_core_runs[k][h][t][0]) for k in range(NC))
            rlp[h][t] = max(128, ((mx + 127) // 128) * 128)
    chunk_tile = []
    chunk_of_ht = {}
    half_sections = []
    c = 0
    for h in range(2):
        h0 = c
        for t in range(TPC):
            nch = rlp[h][t] // 128
            chunk_of_ht[(h, t)] = (c, nch)
            chunk_tile += [t] * nch
            c += nch
        half_sections.append((h0, c - h0))
    NCHUNK = c
    TOTG = NCHUNK * 128

    gcn_calls = []
    for h, (h0, hn) in enumerate(half_sections):
        s = h0 * 128
        end = (h0 + hn) * 128
        while s < end:
            n_ = min(GCALLN, end - s)
            gcn_calls.append((h, s, n_))
            s += n_

    gcn_idx_pc, gcn_dstloc_pc, gcn_coeff_pc = [], [], []
    for k in range(NC):
        lidx = np.zeros(TOTG, np.int16)
        dloc = np.zeros(TOTG, np.float32)
        cofs = np.zeros(TOTG, np.float32)
        for h in range(2):
            for t in range(TPC):
                c0, _ = chunk_of_ht[(h, t)]
                li, dl, ce = per_core_runs[k][h][t]
                s = c0 * 128
                lidx[s:s + len(li)] = li
                dloc[s:s + len(li)] = dl
                cofs[s:s + len(li)] = ce
        gcn_idx_pc.append(_wrap_idx(lidx))
        gcn_dstloc_pc.append(dloc.reshape(NCHUNK, 128).T.copy())
        gcn_coeff_pc.append(cofs.reshape(NCHUNK, 128).T.copy())

    # ---------------- edge-MLP structure (edge-sharded) --------------------
    dstrow = _row_of_node(dst)
    ebatch_all = batch[src].astype(np.float32)
    grp_all = 2 * (srcrow >= HALFR).astype(np.int64) + (dstrow >= HALFR)
    glp = [0] * 4
    orders, counts = [], []
    for k in range(NC):
        e0 = k * EPC
        g_e = grp_all[e0:e0 + EPC]
        order = np.argsort(g_e, kind="stable")
        cnt = np.bincount(g_e, minlength=4)
        orders.append(order)
        counts.append(cnt)
        for g in range(4):
            glp[g] = max(glp[g], ((int(cnt[g]) + ET - 1) // ET) * ET)
    goff = np.concatenate([[0], np.cumsum(glp)]).astype(np.int64)
    EP = int(goff[4])
    NT = EP // ET
    grp_of_tile = []
    for g in range(4):
        grp_of_tile += [g] * (glp[g] // ET)

    mlp_calls = []
    for g in range(4):
        s = int(goff[g])
        while s < goff[g + 1]:
            n_ = min(GCALLE, int(goff[g + 1]) - s)
            mlp_calls.append((s, n_))
            s += n_

    esrc_pc, edst_pc, ebatch_pc, attr_pc, pos_pc = [], [], [], [], []
    ohE_pc = []
    NTB = (NT + 2) // 3
    for k in range(NC):
        e0 = k * EPC
        order, cnt = orders[k], counts[k]
        si = np.zeros(EP, np.int64)
        valid = np.zeros(EP, bool)
        pos_of_local = np.empty(EPC, np.int64)
        cstart = np.cumsum(np.concatenate([[0], cnt]))
        for g in range(4):
            loc = order[cstart[g]:cstart[g + 1]]
            p0 = int(goff[g])
            si[p0:p0 + len(loc)] = e0 + loc
            valid[p0:p0 + len(loc)] = True
            pos_of_local[loc] = p0 + np.arange(len(loc))
        sr = srcrow[si]
        dr = dstrow[si]
        hs = (sr >= HALFR).astype(np.int64)
        hd = (dr >= HALFR).astype(np.int64)
        esrc_pc.append(_wrap_idx((sr - hs * HALFR).astype(np.int16)))
        edst_pc.append(_wrap_idx((dr - hd * HALFR).astype(np.int16)))
        eb = ebatch_all[si].copy()
        eb[~valid] = 0.0
        ebatch_pc.append(eb.astype(bf16).reshape(1, EP))
        ohE_pc.append((eb[None, :] == np.arange(G, dtype=np.float32)[:, None])
                      .astype(bf16))
        ap = np.zeros((128, NTB * ET), np.float32)
        av = ea[si].copy()
        av[~valid] = 0.0
        for t in range(NT):
            b = 32 * (t % 3)
            cb = (t // 3) * ET
            ap[b:b + EDGE_IN, cb:cb + ET] = av[t * ET:(t + 1) * ET].T
        attr_pc.append(ap.astype(bf16))
        pos_pc.append(pos_of_local)

    # ---------------- node-encoder inputs ----------------------------------
    NTA = (NPCP + ET - 1) // ET
    NTAB = (NTA + 2) // 3
    xpk_pc, bval_pc = [], []
    for k in range(NC):
        g0 = k * NPC
        xT = np.zeros((NODE_IN, NPCP), np.float32)
        xT[:, :NPC] = x[g0:g0 + NPC].T
        xp = np.zeros((128, NTAB * ET), np.float32)
        for t in range(NTA):
            b = 32 * (t % 3)
            cb = (t // 3) * ET
            wv = min(ET, NPCP - t * ET)
            xp[b:b + NODE_IN, cb:cb + wv] = xT[:, t * ET:t * ET + wv]
        xpk_pc.append(xp)
        bvflat = np.full(NPCP, -1.0, np.float32)
        bvflat[:NPC] = batch[g0:g0 + NPC].astype(np.float32)
        bval_pc.append(bvflat.reshape(TPC, 128).T.copy())

    meta = dict(
        NCHUNK=NCHUNK, chunk_tile=chunk_tile, chunk_of_ht=chunk_of_ht,
        gcn_calls=gcn_calls, half_sections=half_sections, rlp=rlp,
        EP=EP, NT=NT, NTB=NTB, grp_of_tile=grp_of_tile, mlp_calls=mlp_calls,
        NTA=NTA, NTAB=NTAB, TOTG=TOTG,
    )
    data = dict(
        inv_cnt=inv_cnt, gcn_idx=gcn_idx_pc, gcn_dstloc=gcn_dstloc_pc,
        gcn_coeff=gcn_coeff_pc, esrc=esrc_pc, edst=edst_pc, ebatch=ebatch_pc,
        attr=attr_pc, xpk=xpk_pc, bval=bval_pc, ohE=ohE_pc,
    )
    reasm = dict(pos=pos_pc)
    return meta, data, reasm


def prep_weights(inputs):
    f32 = np.float32
    w = {}

    def rep_small(wname, k_in):
        v = np.zeros((128, 128), f32)
        ww = np.asarray(inputs[wname], f32)
        for b in range(4):
            v[32 * b:32 * b + k_in, :] = ww
        return v

    w["ne1w"] = rep_small("ne1_w", NODE_IN)
    w["ne2w"] = np.asarray(inputs["ne2_w"], f32)
    ne3wc, ne3bc = _center_w(np.asarray(inputs["ne3_w"], f32),
                             np.asarray(inputs["ne3_b"], f32))
    w["ne3wc"] = ne3wc
    w["ne1b"] = np.asarray(inputs["ne1_b"], f32).reshape(128, 1)
    w["ne2b"] = np.asarray(inputs["ne2_b"], f32).reshape(128, 1)
    w["ne3bc"] = ne3bc.reshape(128, 1)
    w["neg"] = np.asarray(inputs["ne_g"], f32).reshape(128, 1)
    w["nebb"] = np.asarray(inputs["ne_bb"], f32).reshape(128, 1)

    w["ee1w"] = rep_small("ee1_w", EDGE_IN).astype(bf16)
    w["ee2w"] = np.asarray(inputs["ee2_w"], f32).astype(bf16)
    ee3wc, ee3bc = _center_w(np.asarray(inputs["ee3_w"], f32),
                             np.asarray(inputs["ee3_b"], f32))
    w["ee3wc"] = ee3wc.astype(bf16)
    w["ee1b"] = np.asarray(inputs["ee1_b"], f32).reshape(128, 1)
    w["ee2b"] = np.asarray(inputs["ee2_b"], f32).reshape(128, 1)
    w["ee3bc"] = ee3bc.reshape(128, 1)
    w["eeg"] = np.asarray(inputs["ee_g"], f32).reshape(128, 1)
    w["eebb"] = np.asarray(inputs["ee_bb"], f32).reshape(128, 1)

    w["g1w"] = np.asarray(inputs["g1_w"], f32)
    w["g2w"] = np.asarray(inputs["g2_w"], f32)
    w["g1b"] = np.asarray(inputs["g1_b"], f32).reshape(128, 1)
    w["g2bb"] = np.tile(np.asarray(inputs["g2_b"], f32)[None, :], (128, 1))

    w["gp1w"] = np.asarray(inputs["gp1_w"], f32)
    gp2wc, gp2bc = _center_w(np.asarray(inputs["gp2_w"], f32),
                             np.asarray(inputs["gp2_b"], f32))
    w["gp2wc"] = gp2wc
    w["gp1b"] = np.asarray(inputs["gp1_b"], f32).reshape(128, 1)
    w["gp2bc"] = gp2bc.reshape(128, 1)
    w["gpg"] = np.asarray(inputs["gp_g"], f32).reshape(128, 1)
    w["gpbb"] = np.asarray(inputs["gp_bb"], f32).reshape(128, 1)

    ep1 = np.asarray(inputs["ep1_w"], f32)
    w["ep1a"] = ep1[0:128].astype(bf16)
    w["ep1b"] = ep1[128:256].astype(bf16)
    w["ep1c"] = ep1[256:384].copy()
    w["ep1d"] = ep1[384:512].astype(bf16)
    w["ep1bias"] = np.asarray(inputs["ep1_b"], f32).reshape(2, 128).T.copy()
    ep2 = np.asarray(inputs["ep2_w"], f32)
    w["ep2w"] = np.concatenate([ep2[0:128], ep2[128:256]], axis=1).astype(bf16)
    w["ep2b"] = np.asarray(inputs["ep2_b"], f32).reshape(128, 1)
    w["ep3w"] = np.asarray(inputs["ep3_w"], f32).astype(bf16)
    w["ep3b"] = np.asarray(inputs["ep3_b"], f32).reshape(64, 1)
    w["ep4w"] = np.asarray(inputs["ep4_w"], f32).astype(bf16)
    w["ep4b"] = np.asarray(inputs["ep4_b"], f32).reshape(1, 1).copy()

    w["iota128"] = np.broadcast_to(np.arange(128, dtype=f32)[None, :],
                                   (128, 128)).copy()
    w["ones1"] = np.ones((1, 128), f32)
    w["ones1h"] = np.ones((1, 64), np.float16)
    w["iotap"] = np.arange(128, dtype=f32).reshape(128, 1).copy()
    w["ones_over_f"] = np.full((128, 128), 1.0 / 128.0, f32)
    return w


# keys that stay HBM-resident or get custom SBUF handling
_NO_CONST = {"gcn_idx", "esrc", "edst", "ohE"}


def build_program(meta, w, data_shapes):
    import os as _os
    PHASE = int(_os.environ.get("K_PHASE", "4"))
    NTLIM = int(_os.environ.get("K_NTLIM", "0"))
    NOGATH = int(_os.environ.get("K_NOGATH", "0"))
    NOPB = int(_os.environ.get("K_NOPB", "0"))
    NOAG = int(_os.environ.get("K_NOAG", "0"))
    NOCONST = int(_os.environ.get("K_NOCONST", "0"))
    import concourse.bacc as bacc
    import concourse.mybir as mybir
    import concourse.tile as tile

    f32, b16, i16 = mybir.dt.float32, mybir.dt.float16, mybir.dt.int16
    AF = mybir.ActivationFunctionType
    ALU = mybir.AluOpType

    NCHUNK, NT, EP, NTB = meta["NCHUNK"], meta["NT"], meta["EP"], meta["NTB"]
    NTA, NTAB, TOTG = meta["NTA"], meta["NTAB"], meta["TOTG"]
    chunk_of_ht = meta["chunk_of_ht"]
    gcn_calls = meta["gcn_calls"]
    mlp_calls = meta["mlp_calls"]
    grp_of_tile = meta["grp_of_tile"]

    nc = bacc.Bacc("TRN2", target_bir_lowering=False, debug=False,
                   num_devices=NC)

    t_in = {}
    for nm, arr in w.items():
        if isinstance(arr, np.ndarray):
            dt = b16 if arr.dtype == bf16 else f32
            t_in[nm] = nc.dram_tensor(nm, list(arr.shape), dt,
                                      kind="ExternalInput")
    for nm, (shape, dt_s) in data_shapes.items():
        dt = {"f32": f32, "b16": b16, "i16": i16}[dt_s]
        t_in[nm] = nc.dram_tensor(nm, list(shape), dt, kind="ExternalInput")

    out_d = nc.dram_tensor("out", [1, EP], f32, kind="ExternalOutput")
    rg = [list(range(NC))]

    with tile.TileContext(nc) as tc:
        from contextlib import ExitStack
        with ExitStack() as ctx:
            cpool = ctx.enter_context(tc.tile_pool(name="consts", bufs=1))
            dram = ctx.enter_context(tc.tile_pool(name="dram", bufs=1,
                                                  space="DRAM"))
            ps = ctx.enter_context(tc.tile_pool(name="ps", bufs=7,
                                                space="PSUM"))
            work = ctx.enter_context(tc.tile_pool(name="work", bufs=2))
            big = ctx.enter_context(tc.tile_pool(name="big", bufs=1))
            efT_pool = ctx.enter_context(tc.tile_pool(name="efT", bufs=10))
            zc_pool = ctx.enter_context(tc.tile_pool(name="zc", bufs=3))

            # ---- constants into SBUF
            c_sb = {}
            for nm, t in t_in.items():
                if nm in _NO_CONST:
                    continue
                tile_ = cpool.tile(list(t.shape), t.dtype, tag=f"c_{nm}")
                if not NOCONST:
                    nc.sync.dma_start(tile_[:], t.ap())
                c_sb[nm] = tile_

            def C(nm):
                return c_sb[nm][:]

            # ---- DRAM scratch
            xw1_own = dram.tile([NPCP, H], b16)
            xw1_full = dram.tile([ROWS, H], b16)
            xw2_own = dram.tile([NPCP, H], b16)
            xw2_full = dram.tile([ROWS, H], b16)
            h2b_own = dram.tile([NPCP, H], b16)
            h2b_full = dram.tile([ROWS, H], b16)
            xw1_b = dram.tile([HALFR, H], b16)
            xw2_b = dram.tile([HALFR, H], b16)
            h2b_b = dram.tile([HALFR, H], b16)
            ar_in = dram.tile([128, G], f32)
            ar_out = dram.tile([128, G], f32)

            # ---- small persistent SBUF
            sums_acc = big.tile([128, G], f32, tag="sums")
            gfT = big.tile([128, G], f32, tag="gfT")
            P_sb = big.tile([64, 256], b16, tag="P")
            esrc_sb = big.tile([128, EP // 16], i16, tag="esrc")
            nc.sync.dma_start(esrc_sb[:], t_in["esrc"].ap())
            edst_sb = big.tile([128, EP // 16], i16, tag="edst")
            nc.sync.dma_start(edst_sb[:], t_in["edst"].ap())

            nc.vector.memset(sums_acc[:], 0.0)

            # ---- LayerNorm tail helper (z centered, [128, wv] f32 in SBUF)
            def ln_tail(lnp, z_ap, wv, gname, bbname, out_ap):
                sq = lnp.tile([128, ET], f32, tag="ln_sq")
                nc.scalar.activation(sq[:, :wv], z_ap, AF.Square)
                # all-ones lhsT -> every output row holds the column mean-sq:
                # the variance arrives already partition-broadcast.
                msp = ps.tile([128, ET], f32, tag="ps")
                nc.tensor.matmul(msp[:, :wv], C("ones_over_f"), sq[:, :wv],
                                 start=True, stop=True)
                msv = lnp.tile([128, ET], f32, tag="ln_ms")
                nc.vector.tensor_scalar(msv[:, :wv], msp[:, :wv], LN_EPS,
                                        None, ALU.add)
                nc.vector.reciprocal(msv[:, :wv], msv[:, :wv])
                rstd = lnp.tile([128, ET], f32, tag="ln_rs")
                nc.scalar.activation(rstd[:, :wv], msv[:, :wv], AF.Sqrt)
                rstdb = lnp.tile([128, ET], f32, tag="ln_rb")
                nc.vector.tensor_mul(rstdb[:, :wv], z_ap, rstd[:, :wv])
                nc.scalar.activation(out_ap, rstdb[:, :wv], AF.Identity,
                                     bias=C(bbname), scale=C(gname))

            # ================= phase A: node encoder + xw1 =================
            if PHASE >= 1:
              with tc.tile_pool(name="pA", bufs=2) as pa, \
                 tc.tile_pool(name="pAbig", bufs=1) as pabig:
                h0T = pabig.tile([128, NPCP], f32, tag="h0T")
                for t in range(NTA):
                    b = 32 * (t % 3)
                    cb = (t // 3) * ET
                    wv = min(ET, NPCP - t * ET)
                    a_ap = c_sb["xpk"][b:b + NODE_IN, cb:cb + wv]
                    z1p = ps.tile([128, ET], f32, tag="ps")
                    nc.tensor.matmul(z1p[:, :wv],
                                     c_sb["ne1w"][b:b + NODE_IN, :], a_ap,
                                     start=True, stop=True)
                    z1s = pa.tile([128, ET], f32, tag="nz1")
                    nc.vector.tensor_scalar(z1s[:, :wv], z1p[:, :wv],
                                            C("ne1b"), 0.0, ALU.add, ALU.max)
                    z2p = ps.tile([128, ET], f32, tag="ps")
                    nc.tensor.matmul(z2p[:, :wv], C("ne2w"), z1s[:, :wv],
                                     start=True, stop=True)
                    z2s = pa.tile([128, ET], f32, tag="nz2")
                    nc.vector.tensor_scalar(z2s[:, :wv], z2p[:, :wv],
                                            C("ne2b"), 0.0, ALU.add, ALU.max)
                    z3p = ps.tile([128, ET], f32, tag="ps")
                    nc.tensor.matmul(z3p[:, :wv], C("ne3wc"), z2s[:, :wv],
                                     start=True, stop=True)
                    z3s = pa.tile([128, ET], f32, tag="nz3")
                    nc.vector.tensor_scalar(z3s[:, :wv], z3p[:, :wv],
                                            C("ne3bc"), None, ALU.add)
                    ln_tail(pa, z3s[:, :wv], wv, "neg", "nebb",
                            h0T[:, t * ET:t * ET + wv])

                for t in range(TPC):
                    xp = ps.tile([128, ET], f32, tag="ps")
                    nc.tensor.matmul(xp[:, :H],
                                     h0T[:, t * 128:(t + 1) * 128],
                                     C("g1w"), start=True, stop=True)
                    xs = work.tile([128, H], b16, tag="xw_sb")
                    nc.vector.tensor_copy(xs[:], xp[:, :H])
                    nc.sync.dma_start(xw1_own[t * 128:(t + 1) * 128, :],
                                      xs[:])
            if not NOAG:
                nc.gpsimd.collective_compute(
                    "AllGather", ALU.bypass, replica_groups=rg,
                    ins=[xw1_own[:]], outs=[xw1_full[:]])
                nc.sync.dma_start(xw1_b[:], xw1_full[HALFR:ROWS, :])

            # ================= GCN layers =================
            call_of_chunk = {}
            for (hcall, s, n_) in gcn_calls:
                call_of_chunk[s // 128] = (hcall, s, n_)

            with tc.tile_pool(name="pB", bufs=1) as pb, \
                 tc.tile_pool(name="gcn_g", bufs=2) as gpool, \
                 tc.tile_pool(name="spool", bufs=4) as spool:
                h1T = pb.tile([128, NPCP], f32, tag="h1T")
                aggA = pb.tile([128, NPCP], f32, tag="aggA")
                gidx_sb = pb.tile([128, TOTG // 16], i16, tag="gidx")
                nc.sync.dma_start(gidx_sb[:], t_in["gcn_idx"].ap())

                def gcn_layer(layer, table_full):
                    cur = {"buf": None, "start": 0}

                    def ensure_gather(c):
                        if c in call_of_chunk:
                            hcall, s, n_ = call_of_chunk[c]
                            gb = gpool.tile([128, GCALLN // 128, H], b16,
                                            tag="gb")
                            view = (table_full[0][0:HALFR, :] if hcall == 0
                                    else table_full[1][:])
                            nc.gpsimd.dma_gather(
                                gb[:, :n_ // 128, :], view,
                                gidx_sb[:, s // 16:(s + n_) // 16],
                                n_, n_, H, single_packet=False)
                            cur["buf"] = gb
                            cur["start"] = c

                    for h in range(2):
                        for t in range(TPC):
                            c0, nch = chunk_of_ht[(h, t)]
                            pst = ps.tile([128, ET], f32, tag="ps")
                            for j in range(nch):
                                c = c0 + j
                                ensure_gather(c)
                                S = spool.tile([128, 128], b16, tag="S")
                                nc.vector.tensor_scalar(
                                    S[:], C("iota128"),
                                    c_sb["gcn_dstloc"][:, c:c + 1],
                                    c_sb["gcn_coeff"][:, c:c + 1],
                                    ALU.is_equal, ALU.mult)
                                gsl = cur["buf"][:, c - cur["start"], :]
                                if layer == 0:
                                    nc.tensor.matmul(pst[:, :128], gsl, S[:],
                                                     start=(j == 0),
                                                     stop=(j == nch - 1))
                                else:
                                    nc.tensor.matmul(pst[:, :128], S[:], gsl,
                                                     start=(j == 0),
                                                     stop=(j == nch - 1))
                            sl = aggA[:, t * 128:(t + 1) * 128]
                            if h == 0:
                                nc.vector.tensor_copy(sl, pst[:, :128])
                                continue
                            nc.vector.tensor_add(sl, sl, pst[:, :128])
                            if layer == 0:
                                nc.scalar.activation(
                                    h1T[:, t * 128:(t + 1) * 128], sl,
                                    AF.Relu, bias=C("g1b"))
                                xp = ps.tile([128, ET], f32, tag="ps")
                                nc.tensor.matmul(
                                    xp[:, :H],
                                    h1T[:, t * 128:(t + 1) * 128],
                                    C("g2w"), start=True, stop=True)
                                xs = work.tile([128, H], b16, tag="xw_sb")
                                nc.vector.tensor_copy(xs[:], xp[:, :H])
                                nc.sync.dma_start(
                                    xw2_own[t * 128:(t + 1) * 128, :], xs[:])
                            else:
                                h2t = work.tile([128, H], f32, tag="h2t")
                                nc.vector.tensor_add(h2t[:], sl, C("g2bb"))
                                ohb = work.tile([128, G], f32, tag="ohb")
                                nc.vector.tensor_scalar(
                                    ohb[:], c_sb["iota128"][:, 0:G],
                                    c_sb["bval"][:, t:t + 1], None,
                                    ALU.is_equal)
                                pp = ps.tile([128, ET], f32, tag="ps")
                                nc.tensor.matmul(pp[:, :G], h2t[:], ohb[:],
                                                 start=True, stop=True)
                                nc.vector.tensor_add(sums_acc[:],
                                                     sums_acc[:], pp[:, :G])
                                h2b = work.tile([128, H], b16, tag="h2b")
                                nc.vector.tensor_copy(h2b[:], h2t[:])
                                nc.sync.dma_start(
                                    h2b_own[t * 128:(t + 1) * 128, :],
                                    h2b[:])

                if PHASE >= 2:
                    gcn_layer(0, (xw1_full, xw1_b))
                if PHASE >= 3:
                    nc.gpsimd.collective_compute(
                        "AllGather", ALU.bypass, replica_groups=rg,
                        ins=[xw2_own[:]], outs=[xw2_full[:]])
                    nc.sync.dma_start(xw2_b[:], xw2_full[HALFR:ROWS, :])
                    gcn_layer(1, (xw2_full, xw2_b))
                    nc.gpsimd.collective_compute(
                        "AllGather", ALU.bypass, replica_groups=rg,
                        ins=[h2b_own[:]], outs=[h2b_full[:]])
                    nc.sync.dma_start(h2b_b[:], h2b_full[HALFR:ROWS, :])

            lnpC = ctx.enter_context(tc.tile_pool(name="lnC", bufs=2))
            if PHASE >= 3:
                # ================= graph MLP (replicated) =================
                nc.sync.dma_start(ar_in[:], sums_acc[:])
                nc.gpsimd.collective_compute(
                    "AllReduce", ALU.add, replica_groups=rg,
                    ins=[ar_in[:]], outs=[ar_out[:]])
                sums_sb = work.tile([128, G], f32, tag="sums_sb")
                nc.sync.dma_start(sums_sb[:], ar_out[:])
                icb = work.tile([128, G], f32, tag="icb")
                nc.gpsimd.partition_broadcast(icb[:], c_sb["inv_cnt"][0:1, :])
                gm = work.tile([128, G], f32, tag="gm")
                nc.vector.tensor_mul(gm[:], sums_sb[:], icb[:])
                z1p = ps.tile([128, ET], f32, tag="ps")
                nc.tensor.matmul(z1p[:, :G], C("gp1w"), gm[:], start=True,
                                 stop=True)
                gf1 = work.tile([128, G], f32, tag="gf1")
                nc.scalar.activation(gf1[:], z1p[:, :G], AF.Relu, bias=C("gp1b"))
                z2p = ps.tile([128, ET], f32, tag="ps")
                nc.tensor.matmul(z2p[:, :G], C("gp2wc"), gf1[:], start=True,
                                 stop=True)
                z2c = work.tile([128, G], f32, tag="z2c")
                nc.vector.tensor_scalar(z2c[:], z2p[:, :G], C("gp2bc"), None,
                                        ALU.add)
                ln_tail(lnpC, z2c[:], G, "gpg", "gpbb", gfT[:])
                Pp = ps.tile([128, ET], f32, tag="ps")
                nc.tensor.matmul(Pp[:64, :256], gfT[:], C("ep1c"), start=True,
                                 stop=True)
                nc.vector.tensor_copy(P_sb[:], Pp[:64, :256])

            if PHASE >= 4:
                # ================= phase C: edge MLP =================
                c_call_of_tile = {}
                for (s, n_) in mlp_calls:
                    c_call_of_tile[s // ET] = (s, n_)

                with tc.tile_pool(name="gsrc", bufs=2) as gs_pool, \
                     tc.tile_pool(name="gdst", bufs=2) as gd_pool, \
                     tc.tile_pool(name="ebt", bufs=3) as eb_pool:
                    cbuf = {"s": None, "d": None, "start": 0}
                    for t in range(NTLIM if NTLIM else NT):
                        grp = grp_of_tile[t]
                        hs, hd = grp >> 1, grp & 1
                        if t in c_call_of_tile:
                            s, n_ = c_call_of_tile[t]
                            gsb = gs_pool.tile([128, 1, GCALLE], b16, tag="gs")
                            gdb = gd_pool.tile([128, 1, GCALLE], b16, tag="gd")
                            vs = (h2b_full[0:HALFR, :] if hs == 0
                                  else h2b_full[HALFR:ROWS, :])
                            vd = (h2b_full[0:HALFR, :] if hd == 0
                                  else h2b_full[HALFR:ROWS, :])
                            if NOGATH:
                                nc.vector.memset(gsb[:], 0.5)
                                nc.vector.memset(gdb[:], 0.5)
                            else:
                                nc.gpsimd.dma_gather(
                                    gsb[:, :, :n_], vs,
                                    esrc_sb[:, s // 16:(s + n_) // 16], n_, n_, H,
                                    transpose=True, single_packet=False)
                                nc.gpsimd.dma_gather(
                                    gdb[:, :, :n_], vd,
                                    edst_sb[:, s // 16:(s + n_) // 16], n_, n_, H,
                                    transpose=True, single_packet=False)
                            cbuf["s"], cbuf["d"] = gsb, gdb
                            cbuf["start"] = s
                        off = t * ET - cbuf["start"]
                        src_sl = cbuf["s"][:, 0, off:off + ET]
                        dst_sl = cbuf["d"][:, 0, off:off + ET]

                        # edge-attr encoder
                        b = 32 * (t % 3)
                        cb = (t // 3) * ET
                        a_ap = c_sb["attr"][b:b + EDGE_IN, cb:cb + ET]
                        z1p = ps.tile([128, ET], f32, tag="ps")
                        nc.tensor.matmul(z1p[:], c_sb["ee1w"][b:b + EDGE_IN, :],
                                         a_ap, start=True, stop=True)
                        z1s = zc_pool.tile([128, ET], b16, tag="ez1")
                        nc.vector.tensor_scalar(z1s[:], z1p[:], C("ee1b"), 0.0,
                                                ALU.add, ALU.max)
                        z2p = ps.tile([128, ET], f32, tag="ps")
                        nc.tensor.matmul(z2p[:], C("ee2w"), z1s[:], start=True,
                                         stop=True)
                        z2s = zc_pool.tile([128, ET], b16, tag="ez2")
                        nc.vector.tensor_scalar(z2s[:], z2p[:], C("ee2b"), 0.0,
                                                ALU.add, ALU.max)
                        z3p = ps.tile([128, ET], f32, tag="ps")
                        nc.tensor.matmul(z3p[:], C("ee3wc"), z2s[:], start=True,
                                         stop=True)
                        z3s = zc_pool.tile([128, ET], f32, tag="ez3")
                        nc.vector.tensor_scalar(z3s[:], z3p[:], C("ee3bc"), None,
                                                ALU.add)
                        eft = efT_pool.tile([128, ET], b16, tag="eft")
                        ln_tail(lnpC, z3s[:], ET, "eeg", "eebb", eft[:])

                        # gf one-hot (host-precomputed), 4 tiles per DMA
                        if t % 4 == 0:
                            ohw = min(4, (NTLIM if NTLIM else NT) - t) * ET
                            oh4 = eb_pool.tile([64, 4 * ET], b16, tag="oht")  # noqa
                            nc.sync.dma_start(
                                oh4[:, :ohw],
                                t_in["ohE"].ap()[0:G, t * ET:t * ET + ohw])
                            cbuf["oh4"] = oh4
                        oh = cbuf["oh4"][:, (t % 4) * ET:(t % 4 + 1) * ET]

                        # L1
                        z1sb = []
                        for mc in range(2):
                            zp = ps.tile([128, ET], f32, tag="ps")
                            m0 = mc * 128
                            nc.tensor.matmul(zp[:], c_sb["ep1a"][:, m0:m0 + 128],
                                             src_sl, start=True, stop=False)
                            nc.tensor.matmul(zp[:], c_sb["ep1b"][:, m0:m0 + 128],
                                             dst_sl, start=False, stop=False)
                            nc.tensor.matmul(zp[:], c_sb["ep1d"][:, m0:m0 + 128],
                                             eft[:], start=False, stop=False)
                            nc.tensor.matmul(zp[:], P_sb[:, m0:m0 + 128],
                                             oh, start=False, stop=True)
                            zs = zc_pool.tile([128, ET], b16, tag=f"z1_{mc}")
                            nc.scalar.activation(
                                zs[:], zp[:], AF.Tanh,
                                bias=c_sb["ep1bias"][:, mc:mc + 1])
                            z1sb.append(zs)

                        # L2
                        z2pp = ps.tile([128, ET], f32, tag="ps")
                        for kc in range(2):
                            nc.tensor.matmul(
                                z2pp[:], c_sb["ep2w"][:, kc * 128:kc * 128 + 128],
                                z1sb[kc][:], start=(kc == 0), stop=(kc == 1))
                        z2sb = zc_pool.tile([128, ET], b16, tag="z2")
                        nc.scalar.activation(z2sb[:], z2pp[:], AF.Tanh,
                                             bias=C("ep2b"))

                        # L3
                        z3pp = ps.tile([128, ET], f32, tag="ps")
                        nc.tensor.matmul(z3pp[:64, :], C("ep3w"), z2sb[:],
                                         start=True, stop=True)
                        z3sb = zc_pool.tile([64, ET], b16, tag="z3")
                        nc.vector.tensor_scalar(z3sb[:], z3pp[:64, :], C("ep3b"),
                                                0.0, ALU.add, ALU.max)

                        # L4 + sigmoid
                        z4p = ps.tile([128, ET], f32, tag="ps")
                        nc.tensor.matmul(z4p[:1, :], C("ep4w"), z3sb[:],
                                         start=True, stop=True)
                        if t % 4 == 0:
                            ob4_t = eb_pool.tile([1, 4 * ET], f32, tag="os4")
                            cbuf["ob4"] = ob4_t
                        ob4 = cbuf["ob4"]
                        nc.scalar.activation(
                            ob4[0:1, (t % 4) * ET:(t % 4 + 1) * ET],
                            z4p[:1, :], AF.Sigmoid, bias=C("ep4b"))
                        ntl = NTLIM if NTLIM else NT
                        if t % 4 == 3 or t == ntl - 1:
                            t0b = (t // 4) * 4
                            wv_o = (t - t0b + 1) * ET
                            nc.sync.dma_start(
                                out_d.ap()[0:1, t0b * ET:t0b * ET + wv_o],
                                ob4[0:1, :wv_o])

    nc.compile()
    return nc


def _data_shapes(meta, data):
    i16, b16s, f32s = "i16", "b16", "f32"
    return {
        "inv_cnt": ([1, G], f32s),
        "xpk": (list(data["xpk"][0].shape), f32s),
        "bval": ([128, TPC], f32s),
        "gcn_idx": (list(data["gcn_idx"][0].shape), i16),
        "gcn_dstloc": ([128, meta["NCHUNK"]], f32s),
        "gcn_coeff": ([128, meta["NCHUNK"]], f32s),
        "esrc": (list(data["esrc"][0].shape), i16),
        "edst": (list(data["edst"][0].shape), i16),
        "ohE": ([G, meta["EP"]], b16s),
        "attr": (list(data["attr"][0].shape), b16s),
    }


def build_all(inputs):
    """Build program + per-core input maps. Shared by kernel() and bench."""
    meta, data, reasm = preprocess(inputs)
    w = prep_weights(inputs)
    nc = build_program(meta, w, _data_shapes(meta, data))

    in_maps = []
    for k in range(NC):
        m = {nm: arr for nm, arr in w.items() if isinstance(arr, np.ndarray)}
        m["inv_cnt"] = data["inv_cnt"].reshape(1, G)
        m["xpk"] = data["xpk"][k]
        m["bval"] = data["bval"][k]
        m["gcn_idx"] = data["gcn_idx"][k]
        m["gcn_dstloc"] = data["gcn_dstloc"][k]
        m["gcn_coeff"] = data["gcn_coeff"][k]
        m["esrc"] = data["esrc"][k]
        m["edst"] = data["edst"][k]
        m["ohE"] = data["ohE"][k]
        m["attr"] = data["attr"][k]
        in_maps.append(m)
    return nc, in_maps, meta, reasm


def kernel(**inputs) -> np.ndarray:
    from concourse.bass_utils import run_bass_kernel_spmd

    nc, in_maps, meta, reasm = build_all(inputs)

    import os as _os0
    _tr = bool(int(_os0.environ.get("K_TRACE", "0")))
    _kw = {}
    if _tr:
        _kw["trace"] = True
        _td = _os0.environ.get("K_TMPDIR")
        if _td:
            _kw["tmpdir"] = _td
        _tc = _os0.environ.get("K_TRACE_CORES")
        if _tc:
            _kw["trace_cores"] = [int(c) for c in _tc.split(",")]
    res = run_bass_kernel_spmd(nc, in_maps, core_ids=list(range(NC)), **_kw)
    globals()["LAST_RESULTS"] = res

    import os as _os, time as _time
    nbench = int(_os.environ.get("K_BENCH", "0"))
    if nbench:
        times = []
        for _ in range(nbench):
            t0 = _time.time()
            run_bass_kernel_spmd(nc, in_maps, core_ids=list(range(NC)))
            times.append(_time.time() - t0)
        globals()["LAST_BENCH"] = times

    out = np.empty((E, 1), np.float32)
    for k in range(NC):
        oc = np.asarray(res.results[k]["out"]).reshape(-1)
        e0 = k * EPC
        out[e0:e0 + EPC, 0] = oc[reasm["pos"][k]]
    return out

